# revision 10
# baseline (speedup 1.0000x reference)
"""Trainium2 Bass kernel for nn_BambaMixerDecoderLayer_84696755077458.

Tensor-parallel over 8 NeuronCores (vLLM-style), v2 (bf16):
  - in_proj / gate_up column-sharded, out_proj / down row-sharded
  - heads + conv channels sharded with d_inner; B/C conv streams replicated
  - SSM scan via chunked SSD (Mamba2): intra-chunk matmuls + small
    cross-chunk state recurrence.
  - bf16 weights/activations for all large GEMMs, scratch and collectives;
    fp32 for stats, decay rows and the SSD state.
  - Single merged in_proj pass; MLP (gate_up+down) fused in one pass.
  - Collectives chunked 8x along tokens and issued inline so they overlap
    with compute (no global barrier between SSD and MLP phases).
Everything on-device is feature-major ([feature, token]); host does layout
transforms (transpose / shard / concat) only.

Self-contained: hardcodes all shapes; needs only /opt/trn_rl_repo on sys.path.
"""
import sys
from contextlib import ExitStack

if '/opt/trn_rl_repo' not in sys.path:
    sys.path.insert(0, '/opt/trn_rl_repo')

import numpy as np

# ---------------------------------------------------------------- constants
H = 2048          # hidden
DIN = 4096        # mamba intermediate
DS = 128          # ssm state
DCONV = 4
NH = 64
HD = 64
FF = 8192
EPS = 1e-5
B, L = 2, 2048
T = B * L                         # 4096 tokens
CONV_DIM = DIN + 2 * DS           # 4352
D_IN_PROJ = 2 * DIN + 2 * DS + NH  # 8512

TP = 8
NHr = NH // TP                    # 8 heads / core
DINr = DIN // TP                  # 512
FFr = FF // TP                    # 1024
CONVr = DINr + 2 * DS             # 768 conv channels / core
MPROJ = DINr + CONVr + NHr        # 1288 in_proj cols / core

Q = 128                           # SSD chunk
NT = 512                          # token tile (also the collective chunk)
NEG = -3.0e38
SIM_SILU = False   # True: emit sigmoid+mul instead of Silu (CoreSim support)


def _f32(x):
    return np.ascontiguousarray(np.asarray(x, dtype=np.float32))


def _bf16(x):
    import ml_dtypes
    return np.ascontiguousarray(
        np.asarray(x, dtype=np.float32).astype(ml_dtypes.bfloat16))


# ================================================================ host prep
def host_constants():
    import ml_dtypes
    identb = np.eye(128, dtype=ml_dtypes.bfloat16)
    i8 = np.eye(8, dtype=np.float32)
    sel8 = np.zeros((8, 8 * 128), np.float32)
    for h in range(8):
        sel8[h, h * 128:(h + 1) * 128] = 1.0
    # negsel4[4g+i, g*512 + i*128 + q'] = -1  (col term for group-of-4 segsum)
    negsel4 = np.zeros((8, 1024), np.float32)
    for h in range(8):
        negsel4[h, h * 128:(h + 1) * 128] = -1.0
    ones8 = np.ones((8, 128), np.float32)
    ones1 = np.ones((1, 128), np.float32)
    ones128 = np.ones((128, 1), np.float32)
    tri = np.where(np.arange(Q)[:, None] > np.arange(Q)[None, :], NEG, 0.0)
    trimask4 = np.concatenate([tri] * 4, axis=1).astype(ml_dtypes.bfloat16)
    return dict(c_identb=identb, c_i8=i8, c_sel8=sel8,
                c_sel8b=sel8.astype(ml_dtypes.bfloat16), c_negsel4=negsel4,
                c_ones8=ones8, c_ones1=ones1, c_ones128=ones128,
                c_trimask4=trimask4)


def shard_core_inputs(inputs, r):
    """Build the per-core input map (all feature-major)."""
    w_in = _f32(inputs['w_in'])
    zs = slice(DINr * r, DINr * (r + 1))
    xs = slice(DIN + DINr * r, DIN + DINr * (r + 1))
    bs = slice(2 * DIN, 2 * DIN + DS)
    cs = slice(2 * DIN + DS, 2 * DIN + 2 * DS)
    dts = slice(2 * DIN + 2 * DS + NHr * r, 2 * DIN + 2 * DS + NHr * (r + 1))
    w_in_r = np.concatenate(
        [w_in[:, zs], w_in[:, xs], w_in[:, bs], w_in[:, cs], w_in[:, dts]], axis=1)

    conv_w = _f32(inputs['conv_w'])
    conv_w_r = np.concatenate([conv_w[DINr * r:DINr * (r + 1)], conv_w[DIN:]], axis=0)
    conv_b = _f32(inputs['conv_b'])
    conv_b_r = np.concatenate([conv_b[DINr * r:DINr * (r + 1)], conv_b[DIN:]], axis=0)

    hs = _f32(inputs['hidden_states'])
    hs = hs.reshape(-1, H)

    A_r = _f32(inputs['A_log'])[NHr * r:NHr * (r + 1)]
    dtb_r = _f32(inputs['dt_bias'])[NHr * r:NHr * (r + 1)]
    D_r = _f32(inputs['D_ssm'])[NHr * r:NHr * (r + 1)]

    m = dict(host_constants())
    m['hsT'] = _bf16(hs.T)                                      # [2048, T]
    m['w_in'] = _bf16(w_in_r)                                   # [2048, 1288]
    # per-k-tile column form of per-feature vectors: [128, n_tiles]
    m['ln1_c'] = np.ascontiguousarray(_f32(inputs['ln1_w']).reshape(16, 128).T)
    m['ln2_c'] = np.ascontiguousarray(_f32(inputs['ln2_w']).reshape(16, 128).T)
    m['normw_c'] = np.ascontiguousarray(
        _f32(inputs['norm_w'])[DINr * r:DINr * (r + 1)].reshape(4, 128).T)
    m['dssm_c'] = np.ascontiguousarray(
        np.repeat(D_r, HD).reshape(4, 128).T)                   # [128, 4]
    # conv weights: [128, 6*4] with [p, pt*4+d]
    m['conv_w'] = np.ascontiguousarray(
        conv_w_r.reshape(6, 128, DCONV).transpose(1, 0, 2).reshape(128, 6 * DCONV))
    m['conv_b'] = np.ascontiguousarray(conv_b_r.reshape(6, 128).T)  # [128, 6]
    m['a_col'] = np.ascontiguousarray((-np.exp(A_r))[:, None])   # [8,1]
    m['dtb_col'] = np.ascontiguousarray(dtb_r[:, None])          # [8,1]
    m['w_out'] = _bf16(_f32(inputs['w_out'])[DINr * r:DINr * (r + 1)])
    wgu = _f32(inputs['w_gate_up'])
    m['w_gate'] = _bf16(wgu[:, FFr * r:FFr * (r + 1)])
    m['w_up'] = _bf16(wgu[:, FF + FFr * r:FF + FFr * (r + 1)])
    m['w_down'] = _bf16(_f32(inputs['w_down'])[FFr * r:FFr * (r + 1)])
    return m


# ================================================================ the kernel
def build(world=TP, debug=False, T_=T):
    import concourse.mybir as mybir
    import concourse.tile as tile
    from concourse import bacc
    from concourse.alu_op_type import AluOpType as Op

    AF = mybir.ActivationFunctionType
    F32 = mybir.dt.float32
    BF16 = mybir.dt.bfloat16

    nc = bacc.Bacc("TRN2", target_bir_lowering=False, debug=False,
                   num_devices=world)

    F32R = mybir.dt.float32r
    n8 = T_ // NT

    def din(name, shape, dt):
        return nc.dram_tensor(name, list(shape), dt, kind="ExternalInput").ap()

    BIN = {'hsT', 'w_in', 'w_out', 'w_gate', 'w_up', 'w_down', 'c_identb',
           'c_trimask4', 'c_sel8b'}
    RIN = {'c_i8', 'c_sel8', 'c_negsel4', 'c_ones8', 'c_ones1', 'c_ones128'}
    io = {}
    for name, shape in [
            ('hsT', (H, T_)), ('w_in', (H, MPROJ)),
            ('ln1_c', (128, 16)), ('ln2_c', (128, 16)),
            ('normw_c', (128, 4)), ('dssm_c', (128, 4)),
            ('conv_w', (128, 24)), ('conv_b', (128, 6)),
            ('a_col', (8, 1)), ('dtb_col', (8, 1)),
            ('w_out', (DINr, H)), ('w_gate', (H, FFr)), ('w_up', (H, FFr)),
            ('w_down', (FFr, H)),
            ('c_identb', (128, 128)), ('c_i8', (8, 8)), ('c_sel8', (8, 1024)),
            ('c_sel8b', (8, 1024)), ('c_negsel4', (8, 1024)), ('c_ones8', (8, 128)),
            ('c_ones1', (1, 128)), ('c_ones128', (128, 1)),
            ('c_trimask4', (128, 512))]:
        dt = BF16 if name in BIN else (F32R if name in RIN else F32)
        io[name] = din(name, shape, dt)

    io['out1T'] = nc.dram_tensor("out1T", [H // world, T_], BF16,
                                 kind="ExternalOutput").ap()
    io['resid2T'] = nc.dram_tensor("resid2T", [H, T_], BF16,
                                   kind="ExternalOutput").ap()

    skind = "ExternalOutput" if debug else "Internal"
    scr = {}
    scr['z'] = nc.dram_tensor("z_s", [DINr, T_], BF16, kind=skind).ap()
    scr['x'] = nc.dram_tensor("x_s", [DINr, T_], BF16, kind=skind).ap()
    scr['b'] = nc.dram_tensor("b_s", [DS, T_], BF16, kind=skind).ap()
    scr['c'] = nc.dram_tensor("c_s", [DS, T_], BF16, kind=skind).ap()
    scr['ar1_in8'] = [
        nc.dram_tensor(f"ar1_in{q}", [H, NT], BF16, kind="Internal").ap()
        for q in range(n8)]
    scr['ar1_out8'] = [
        nc.dram_tensor(f"ar1_out{q}", [H, NT], BF16, kind="Internal",
                       addr_space="Shared").ap() for q in range(n8)]
    scr['ssq_in8'] = [
        nc.dram_tensor(f"ssq_in{q}", [1, NT], F32, kind="Internal").ap()
        for q in range(n8)]
    scr['ssq_out8'] = [
        nc.dram_tensor(f"ssq_out{q}", [1, NT], F32, kind="Internal",
                       addr_space="Shared").ap() for q in range(n8)]
    scr['rs2_in8'] = [
        nc.dram_tensor(f"rs2_in{q}", [H, NT], BF16, kind="Internal").ap()
        for q in range(n8)]
    scr['rs2_out8'] = [
        nc.dram_tensor(f"rs2_out{q}", [H // world, NT], BF16,
                       kind="Internal").ap() for q in range(n8)]
    scr['mtn0'] = nc.dram_tensor("mtn0_s", [H, NT], BF16, kind="Internal").ap()
    scr['rs2h_in'] = [
        nc.dram_tensor(f"rs2h_in{q}", [H, NT // 4], BF16,
                       kind="Internal").ap() for q in range(4)]
    scr['rs2h_out'] = [
        nc.dram_tensor(f"rs2h_out{q}", [H // world, NT // 4], BF16,
                       kind="Internal").ap() for q in range(4)]

    with tile.TileContext(nc) as tc:
        _body(tc, io, scr, world, debug, mybir, tile, Op, AF, F32, T_)

    nc.compile()
    return nc


def _body(tc, io, scr, world, debug, mybir, tile, Op, AF, F32, T_):
    nc = tc.nc
    F32R = mybir.dt.float32r
    BF16 = mybir.dt.bfloat16
    n8 = T_ // NT
    NCHUNK = T_ // Q
    CPS = (T_ // B) // Q          # chunks per sequence

    def mm(out, lhsT, rhs, start, stop, skip=False):
        if lhsT.dtype == F32:
            lhsT = lhsT.bitcast(F32R)
        if rhs.dtype == F32:
            rhs = rhs.bitcast(F32R)
        nc.tensor.matmul(out, lhsT, rhs, start=start, stop=stop,
                         skip_group_check=skip)

    def silu(out_ap, in_ap, bias=0.0, pool=None, tag="silu_tmp"):
        if SIM_SILU:
            tmp = pool.tile(list(out_ap.shape), F32, tag=tag, name=tag)
            nc.scalar.activation(tmp[:], in_ap, AF.Sigmoid, bias=bias, scale=1.0)
            if isinstance(bias, float) and bias == 0.0:
                nc.vector.tensor_tensor(out_ap, in_ap, tmp[:], Op.mult)
            else:
                raise NotImplementedError("SIM_SILU with bias AP")
        else:
            nc.scalar.activation(out_ap, in_ap, AF.Silu, bias=bias, scale=1.0)

    def allreduce(in_ap, out_ap):
        if world > 1:
            nc.gpsimd.collective_compute(
                "AllReduce", Op.add, replica_groups=[list(range(world))],
                ins=[in_ap], outs=[out_ap])
        else:
            nc.sync.dma_start(out_ap, in_ap)

    with ExitStack() as ES:
        cpool = ES.enter_context(tc.tile_pool(name="consts", bufs=1))

        # -------------------------------------------------------- constants
        C = {}
        RT = {'c_i8', 'c_sel8', 'c_negsel4', 'c_ones8', 'c_ones1',
              'c_ones128'}
        BT = {'c_identb', 'c_trimask4', 'c_sel8b'}
        for nm, shape in [('c_identb', (128, 128)), ('c_i8', (8, 8)),
                          ('c_sel8', (8, 1024)), ('c_sel8b', (8, 1024)),
                          ('c_negsel4', (8, 1024)),
                          ('c_ones8', (8, 128)), ('c_ones1', (1, 128)),
                          ('c_ones128', (128, 1)), ('c_trimask4', (128, 512)),
                          ('ln1_c', (128, 16)), ('ln2_c', (128, 16)),
                          ('normw_c', (128, 4)), ('dssm_c', (128, 4)),
                          ('conv_w', (128, 24)), ('conv_b', (128, 6)),
                          ('a_col', (8, 1)), ('dtb_col', (8, 1))]:
            dt = BF16 if nm in BT else (F32R if nm in RT else F32)
            t = cpool.tile(list(shape), dt, tag=nm)
            nc.sync.dma_start(t[:], io[nm])
            C[nm] = t
        identb, i8 = C['c_identb'], C['c_i8']
        sel8, negsel4 = C['c_sel8'], C['c_negsel4']
        sel8b = C['c_sel8b']
        ones8, ones1, ones128 = C['c_ones8'], C['c_ones1'], C['c_ones128']
        trimask4 = C['c_trimask4']

        eps1 = cpool.tile([1, 1], F32, tag="eps1", name="eps1")
        nc.vector.memset(eps1[:], float(EPS))

        # ======================================================== Phase 1
        # merged single pass over hsT: ln1 stats + z + dt + xBC + conv
        rows_a_es = ExitStack()
        rows_a = rows_a_es.enter_context(tc.tile_pool(name="rows_a", bufs=1))
        dt_rows = rows_a.tile([8, T_], F32R, tag="dt_rows", name="dt_rows")
        lA_rows = rows_a.tile([8, T_], F32R, tag="lA_rows", name="lA_rows")
        ssq_yz_row = rows_a.tile([1, T_], F32, tag="ssq_yz", name="ssq_yz")

        with tc.tile_pool(name="p1w", bufs=1) as p1w, \
             tc.tile_pool(name="p1", bufs=2) as p1, \
             tc.tile_pool(name="convp", bufs=2) as convp, \
             tc.tile_pool(name="p1ps_s", bufs=1, space="PSUM") as p1ps_s, \
             tc.tile_pool(name="p1ps_m", bufs=2, space="PSUM") as p1ps_m:

            # first token tile is prefetched BEFORE the weights so the ln1
            # stats matmuls warm up the PE while w_in streams in
            hst0 = p1.tile([128, 16, NT], BF16, tag="hst", name="hst")
            nc.sync.dma_start(hst0[:, 0:8, :], io['hsT'][0:8 * 128, 0:NT]
                              .rearrange("(kt p) n -> p kt n", p=128))
            nc.sync.dma_start(hst0[:, 8:16, :], io['hsT'][8 * 128:H, 0:NT]
                              .rearrange("(kt p) n -> p kt n", p=128))
            # all in_proj columns per core: [z | xBC | dt] = 1288
            w1 = p1w.tile([128, 16, MPROJ], BF16, tag="w1", name="w1")
            nc.sync.dma_start(
                w1[:], io['w_in'].rearrange("(kt p) m -> p kt m", p=128))
            for k in range(16):
                nc.vector.tensor_scalar_mul(w1[:, k, :], w1[:, k, :],
                                            C['ln1_c'][:, k:k + 1])

            halo_prev = None
            for nt in range(n8):
                tok0 = nt * NT
                seq_start = (tok0 % (T_ // B)) == 0
                if nt == 0:
                    hst = hst0
                else:
                    hst = p1.tile([128, 16, NT], BF16, tag="hst", name="hst")
                    nc.sync.dma_start(hst[:], io['hsT'][:, tok0:tok0 + NT]
                                      .rearrange("(kt p) n -> p kt n", p=128))
                # ln1 stats (ACT squares; matmuls never wait on these)
                ssq_ps = p1ps_s.tile([1, NT], F32, tag="ssq", name="ssq")
                for k in range(16):
                    sq = p1.tile([128, NT], F32R, tag="sq", name="sq")
                    nc.scalar.activation(sq[:], hst[:, k, :], AF.Square)
                    mm(ssq_ps[:], ones128[:], sq[:],
                       start=(k == 0), stop=(k == 15))
                sr0 = p1.tile([1, NT], F32, tag="sr0", name="sr0", bufs=1)
                nc.scalar.activation(sr0[:], ssq_ps[:], AF.Ln,
                                     bias=eps1[:], scale=float(1.0 / H))
                s_row = p1.tile([1, NT], F32R, tag="s_row", name="s_row",
                                bufs=1)
                nc.scalar.activation(s_row[:], sr0[:], AF.Exp, scale=-0.5)
                sb_ps = p1ps_s.tile([128, NT], F32, tag="sbps", name="sbps")
                mm(sb_ps[:], ones1[:], s_row[:], start=True, stop=True)
                sb = p1.tile([128, NT], F32, tag="sb", name="sb")
                nc.any.tensor_copy(sb[:], sb_ps[:])
                # z m-tiles: matmul on RAW hidden, scale on the way out
                for mi in range(4):
                    ps = p1ps_m.tile([128, NT], F32, tag="mt", name="mt")
                    for k in range(16):
                        mm(ps[:], w1[:, k, mi * 128:(mi + 1) * 128],
                           hst[:, k, :], start=(k == 0), stop=(k == 15))
                    zt = p1.tile([128, NT], BF16, tag="z", name="z")
                    nc.vector.tensor_tensor(zt[:], ps[:], sb[:], Op.mult)
                    nc.sync.dma_start(
                        scr['z'][mi * 128:(mi + 1) * 128, tok0:tok0 + NT], zt[:])
                # dt m-tile (8 wide)
                dtp = p1ps_s.tile([8, NT], F32, tag="mtdt", name="mtdt")
                for k in range(16):
                    mm(dtp[:], w1[:, k, DINr + CONVr:MPROJ], hst[:, k, :],
                       start=(k == 0), stop=(k == 15))
                dt_raw = p1.tile([8, NT], F32, tag="dtraw", name="dtraw",
                                 bufs=1)
                nc.vector.tensor_tensor(dt_raw[:], dtp[:], sb[:8, :], Op.mult)
                e8 = p1.tile([8, NT], F32, tag="e8", name="e8", bufs=1)
                nc.scalar.activation(e8[:], dt_raw[:], AF.Exp,
                                     bias=C['dtb_col'][:], scale=1.0)
                e8p = p1.tile([8, NT], F32, tag="e8p", name="e8p", bufs=1)
                nc.vector.tensor_scalar_add(e8p[:], e8[:], 1.0)
                nc.scalar.activation(dt_rows[:, tok0:tok0 + NT], e8p[:], AF.Ln)
                logda = p1.tile([8, NT], F32, tag="logda", name="logda",
                                bufs=1)
                nc.vector.tensor_scalar_mul(logda[:], dt_rows[:, tok0:tok0 + NT],
                                            C['a_col'][:])
                for c in range(NT // Q):
                    nc.vector.tensor_tensor_scan(
                        lA_rows[:, tok0 + c * Q:tok0 + (c + 1) * Q],
                        ones8[:, :Q].bitcast(F32), logda[:, c * Q:(c + 1) * Q],
                        0.0, Op.mult, Op.add)
                # xBC m-tiles + causal conv
                halo = [convp.tile([128, NT + 3], BF16, tag=f"halo{pt}",
                                   name=f"halo{pt}") for pt in range(6)]
                for pt in range(6):
                    ps = p1ps_m.tile([128, NT], F32, tag="mt", name="mt")
                    for k in range(16):
                        mm(ps[:], w1[:, k, DINr + pt * 128:DINr + (pt + 1) * 128],
                           hst[:, k, :], start=(k == 0), stop=(k == 15))
                    nc.vector.tensor_tensor(halo[pt][:, 3:3 + NT], ps[:], sb[:],
                                            Op.mult)
                for pt in range(6):
                    if seq_start:
                        nc.vector.memset(halo[pt][:, 0:3], 0.0)
                    else:
                        nc.vector.tensor_copy(halo[pt][:, 0:3],
                                              halo_prev[pt][:, NT:NT + 3])
                    acc = convp.tile([128, NT], BF16, tag="cacc", name="cacc")
                    nc.vector.tensor_scalar_mul(
                        acc[:], halo[pt][:, 0:NT],
                        C['conv_w'][:, pt * 4:pt * 4 + 1])
                    for d in range(1, 4):
                        nc.vector.scalar_tensor_tensor(
                            acc[:], halo[pt][:, d:d + NT],
                            C['conv_w'][:, pt * 4 + d:pt * 4 + d + 1],
                            acc[:], Op.mult, Op.add)
                    cact = convp.tile([128, NT], BF16, tag="cact", name="cact")
                    if SIM_SILU:
                        nc.vector.tensor_scalar_add(acc[:], acc[:],
                                                    C['conv_b'][:, pt:pt + 1])
                        silu(cact[:], acc[:], pool=convp, tag="cvsig")
                    else:
                        nc.scalar.activation(cact[:], acc[:], AF.Silu,
                                             bias=C['conv_b'][:, pt:pt + 1],
                                             scale=1.0)
                    if pt < 4:
                        nc.sync.dma_start(
                            scr['x'][pt * 128:(pt + 1) * 128, tok0:tok0 + NT],
                            cact[:])
                    elif pt == 4:
                        nc.sync.dma_start(scr['b'][:, tok0:tok0 + NT], cact[:])
                    else:
                        nc.sync.dma_start(scr['c'][:, tok0:tok0 + NT], cact[:])
                halo_prev = halo

        # ============================================ Phase 2: SSD + gated
        # norm + out_proj, fused per token-tile. out_proj runs on UNSCALED
        # yz — the rms scale s3 commutes through the matmul and the
        # AllReduce, and is applied in Phase 4. AR chunks issued inline.
        with tc.tile_pool(name="p2", bufs=3) as p2, \
             tc.tile_pool(name="p2s", bufs=2) as p2s, \
             tc.tile_pool(name="state", bufs=1) as spool, \
             tc.tile_pool(name="p2pre", bufs=1) as p2pre, \
             tc.tile_pool(name="p3f", bufs=2) as p3f, \
             tc.tile_pool(name="p3w", bufs=1) as p3w, \
             tc.tile_pool(name="p2ps1", bufs=1, space="PSUM") as p2ps1, \
             tc.tile_pool(name="p2ps2", bufs=1, space="PSUM") as p2ps2, \
             tc.tile_pool(name="p3ps", bufs=2, space="PSUM") as p3ps:

            w_out_t = p3w.tile([128, 4, H], BF16, tag="w_out_t", name="w_out_t")
            nc.sync.dma_start(w_out_t[:],
                              io['w_out'].rearrange("(kt p) m -> p kt m", p=128))
            for k in range(4):
                nc.vector.tensor_scalar_mul(w_out_t[:, k, :], w_out_t[:, k, :],
                                            C['normw_c'][:, k:k + 1])

            S_all = spool.tile([128, NHr * HD], F32R, tag="S_all", name="S_all")
            nc.vector.memset(S_all[:].bitcast(F32), 0.0)
            # bf16 shadow of the state for the y_inter matmul (PE cannot mix
            # f32r weights with bf16 moving); fp32 master stays exact
            S_b = spool.tile([128, NHr * HD], BF16, tag="S_b", name="S_b")
            nc.vector.memset(S_b[:], 0.0)

            for nt in range(n8):
                y_sb = p3f.tile([128, 4, NT], BF16, tag="ysb", name="ysb")
                for cc_ in range(NT // Q):
                    ch = nt * (NT // Q) + cc_
                    t0 = ch * Q
                    xf = p2.tile([128, 4, Q], BF16, tag="xf", name="xf")
                    nc.sync.dma_start(xf[:], scr['x'][:, t0:t0 + Q]
                                      .rearrange("(pt p) n -> p pt n", p=128))
                    bf = p2.tile([128, Q], BF16, tag="bf", name="bf")
                    nc.sync.dma_start(bf[:], scr['b'][:, t0:t0 + Q])
                    cf = p2.tile([128, Q], BF16, tag="cf", name="cf")
                    nc.sync.dma_start(cf[:], scr['c'][:, t0:t0 + Q])

                    lrow = lA_rows[:, t0:t0 + Q]
                    dtrow = dt_rows[:, t0:t0 + Q]

                    # exp(lA) rows: bf16 copy feeds the equad broadcasts;
                    # last column stays fp32 (chunk decay for the state).
                    explb = p2s.tile([8, Q], BF16, tag="explb", name="explb")
                    nc.scalar.activation(explb[:], lrow, AF.Exp)
                    expl_l = p2s.tile([8, 1], F32, tag="expl_l", name="expl_l")
                    nc.scalar.activation(expl_l[:], lrow[:, Q - 1:Q], AF.Exp)
                    ddr0 = p2s.tile([8, Q], F32, tag="ddr0", name="ddr0")
                    nc.vector.tensor_scalar(ddr0[:], lrow, -1.0,
                                            lrow[:, Q - 1:Q].bitcast(F32),
                                            Op.mult, Op.add)
                    dd_rows = p2s.tile([8, Q], F32R, tag="ddrows", name="ddrows")
                    nc.scalar.activation(dd_rows[:], ddr0[:], AF.Exp)
                    nc.vector.tensor_tensor(dd_rows[:], dd_rows[:], dtrow,
                                            Op.mult)
                    dg = p2s.tile([8, 8], F32R, tag="dg", name="dg")
                    nc.vector.tensor_scalar_mul(dg[:], i8[:], expl_l[:])

                    misc = p2ps1.tile([128, 160], F32, tag="misc", name="misc")
                    g_ps = misc[:, 0:128]
                    ddcol_ps = misc[:, 128:136]
                    decay_ps = misc[:, 136:144]
                    dtcol_ps = misc[:, 144:152]

                    mm(g_ps, bf[:], cf[:], start=True, stop=True)
                    mm(ddcol_ps, dd_rows[:], i8[:], start=True, stop=True)
                    mm(decay_ps, ones8[:], dg[:], start=True, stop=True)
                    mm(dtcol_ps, dtrow, i8[:], start=True, stop=True)
                    # late-read scalars leave PSUM early so misc can
                    # single-buffer without serializing chunks
                    g_sb = p2s.tile([128, 128], BF16, tag="g_sb", name="g_sb")
                    nc.any.tensor_copy(g_sb[:], g_ps)
                    dsc = p2s.tile([128, 16], F32, tag="dsc", name="dsc")
                    nc.any.tensor_copy(dsc[:], misc[:, 128:144])
                    ddcol_sb = dsc[:, 0:8]
                    decay_sb = dsc[:, 8:16]
                    dtc_b = p2s.tile([128, 8], BF16, tag="dtc_b", name="dtc_b")
                    nc.any.tensor_copy(dtc_b[:], dtcol_ps)

                    tps = p2ps1.tile([128, 5, 128], BF16, tag="xtm", name="xtm")
                    nc.tensor.transpose(tps[:, 4, :], bf[:], identb[:])
                    btm = p2s.tile([128, Q], BF16, tag="btm", name="btm")
                    nc.any.tensor_copy(btm[:], tps[:, 4, :])

                    for pt in range(4):
                        nc.tensor.transpose(tps[:, pt, :],
                                            xf[:, pt, :], identb[:])
                    xtm = p2s.tile([128, NHr, HD], BF16, tag="xtm_sb",
                                   name="xtm_sb")
                    nc.any.tensor_copy(
                        xtm[:], tps[:, 0:4, :].rearrange(
                            "p f (h d) -> p (f h) d", d=HD))
                    xw = p2s.tile([128, NHr, HD], BF16, tag="xw", name="xw")
                    nc.vector.tensor_tensor(
                        xw[:], xtm[:],
                        ddcol_sb[:, :, None].broadcast_to([128, NHr, HD]),
                        Op.mult)

                    # group-of-4 segsum: per-head broadcasts + batched col
                    # and tri-mask accumulated in PSUM fp32 (exact diagonal)
                    for g in range(2):
                        seg = p2ps2.tile([128, 2, 4, 128], F32, tag="seg",
                                         name="seg")
                        # PSUM zero-region semantics: start=True re-marks the
                        # whole 2KB bank pending-zero, so exactly ONE start per
                        # bank; later first-touch writes overwrite via pending
                        # bits and full-width writes accumulate.
                        for i in range(4):
                            h = 4 * g + i
                            mm(seg[:, 0, i, :],
                               sel8[:, h * 128:(h + 1) * 128], lrow,
                               start=(i == 0), stop=False, skip=True)
                        mm(seg[:, 0], lrow,
                           negsel4[:, g * 512:(g + 1) * 512],
                           start=False, stop=False, skip=True)
                        mm(seg[:, 0], identb[:], trimask4[:],
                           start=False, stop=True, skip=True)
                        for i in range(4):
                            h = 4 * g + i
                            mm(seg[:, 1, i, :],
                               sel8b[:, h * 128:(h + 1) * 128], explb[:],
                               start=(i == 0), stop=(i == 3), skip=True)
                        w0 = p2s.tile([128, 4, 128], BF16, tag="w0", name="w0")
                        nc.scalar.activation(w0[:], seg[:, 0], AF.Exp)
                        wt = p2s.tile([128, 4, 128], BF16, tag="wt", name="wt")
                        nc.vector.tensor_tensor(
                            wt[:], w0[:],
                            dtc_b[:, 4 * g:4 * g + 4, None]
                            .broadcast_to([128, 4, 128]), Op.mult)
                        nc.vector.tensor_tensor(
                            wt[:], wt[:],
                            g_sb[:, None, :].broadcast_to([128, 4, 128]),
                            Op.mult)
                        ce = p2s.tile([128, 4, 128], BF16, tag="ce", name="ce")
                        nc.vector.tensor_tensor(
                            ce[:], seg[:, 1],
                            cf[:, None, :].broadcast_to([128, 4, 128]),
                            Op.mult)
                        # y: head 4g+2j   -> partitions 0:64,  col block j
                        #    head 4g+2j+1 -> partitions 64:128, col block j
                        y_cur = p2ps1.tile([128, 256], F32, tag="y", name="y")
                        for i in range(4):
                            h = 4 * g + i
                            j, half = i // 2, i % 2
                            ysl = y_cur[64 * half:64 * (half + 1),
                                        j * 128:(j + 1) * 128]
                            mm(ysl, xtm[:, h, :], wt[:, i, :],
                               start=True, stop=False, skip=True)
                            mm(ysl, S_b[:, h * HD:(h + 1) * HD],
                               ce[:, i, :], start=False, stop=True, skip=True)
                        for j in range(2):
                            pt = 2 * g + j
                            nc.vector.scalar_tensor_tensor(
                                y_sb[:, pt, cc_ * Q:(cc_ + 1) * Q],
                                xf[:, pt, :], C['dssm_c'][:, pt:pt + 1],
                                y_cur[:, j * 128:(j + 1) * 128],
                                Op.mult, Op.add)

                    tp_ps = p2ps1.tile([128, 512], F32, tag="tp", name="tp")
                    mm(tp_ps[:], btm[:], xw[:], start=True, stop=True)
                    S3 = S_all[:].rearrange("p (h d) -> p h d", d=HD)
                    nc.vector.tensor_tensor(
                        S3, S3,
                        decay_sb[:, :, None].broadcast_to([128, NHr, HD])
                        .bitcast(F32), Op.mult)
                    nc.vector.tensor_tensor(
                        S3, S3,
                        tp_ps[:].rearrange("p (h d) -> p h d", d=HD),
                        Op.add)

                    if (ch + 1) % CPS == 0 and ch + 1 < NCHUNK:
                        nc.vector.memset(S_all[:].bitcast(F32), 0.0)
                    if ch + 1 < NCHUNK:
                        nc.any.tensor_copy(S_b[:], S_all[:])

                # gated product + stats + out_proj for this token tile
                tok0 = nt * NT
                zt = p3f.tile([128, 4, NT], BF16, tag="zt", name="zt")
                nc.sync.dma_start(zt[:], scr['z'][:, tok0:tok0 + NT]
                                  .rearrange("(pt p) n -> p pt n", p=128))
                yz_all = p3f.tile([128, 4, NT], BF16, tag="yzall", name="yzall")
                ssq_full = p3ps.tile([128, NT], F32, tag="mt3", name="mt3ssq")
                ssq_ps = ssq_full[0:1, :]
                for pt in range(4):
                    sz = p3f.tile([128, NT], BF16, tag="sz", name="sz")
                    silu(sz[:], zt[:, pt, :], pool=p3f, tag="szsig")
                    nc.vector.tensor_tensor(yz_all[:, pt, :], y_sb[:, pt, :],
                                            sz[:], Op.mult)
                    sqz = p3f.tile([128, NT], F32R, tag="sqz", name="sqz")
                    nc.scalar.activation(sqz[:], yz_all[:, pt, :], AF.Square)
                    mm(ssq_ps[:], ones128[:], sqz[:],
                       start=(pt == 0), stop=(pt == 3))
                nc.any.tensor_copy(ssq_yz_row[:, tok0:tok0 + NT], ssq_ps[:])

                for mi in range(16):
                    ps = p3ps.tile([128, NT], F32, tag="mt3", name="mt3")
                    for k in range(4):
                        mm(ps[:], w_out_t[:, k, mi * 128:(mi + 1) * 128],
                           yz_all[:, k, :], start=(k == 0), stop=(k == 3))
                    ot = p3f.tile([128, NT], BF16, tag="ot", name="ot")
                    nc.any.tensor_copy(ot[:], ps[:])
                    nc.sync.dma_start(
                        scr['ar1_in8'][nt][mi * 128:(mi + 1) * 128, :], ot[:])

                # inline chunked collectives: tiny stats AR then the big AR
                nc.sync.dma_start(scr['ssq_in8'][nt],
                                  ssq_yz_row[:, tok0:tok0 + NT])
                allreduce(scr['ssq_in8'][nt], scr['ssq_out8'][nt])
                allreduce(scr['ar1_in8'][nt], scr['ar1_out8'][nt])

                if nt == max(0, n8 - 3):
                    # Phase-4 prep for tile 0, overlapped with the P2 tail.
                    # Result (mtn for tile 0) is bounced via DRAM.
                    mt0 = p2pre.tile([128, 16, NT], BF16, tag="mt0",
                                     name="mt0")
                    nc.sync.dma_start(mt0[:], scr['ar1_out8'][0]
                                      .rearrange("(kt p) n -> p kt n", p=128))
                    sqt0 = p2s.tile([1, NT], F32, tag="sqt0", name="sqt0")
                    nc.sync.dma_start(sqt0[:], scr['ssq_out8'][0])
                    sql0 = p2s.tile([1, NT], F32, tag="sql0", name="sql0")
                    nc.scalar.activation(sql0[:], sqt0[:], AF.Ln,
                                         bias=eps1[:], scale=float(1.0 / DIN))
                    s3r0 = p2s.tile([1, NT], F32R, tag="s3r0", name="s3r0")
                    nc.scalar.activation(s3r0[:], sql0[:], AF.Exp, scale=-0.5)
                    bps = p3ps.tile([128, NT], F32, tag="mt3", name="mt3s3b")
                    mm(bps[:], ones1[:], s3r0[:], start=True, stop=True)
                    s3b0 = p2pre.tile([128, NT], BF16, tag="s3b0", name="s3b0")
                    nc.any.tensor_copy(s3b0[:], bps[:])
                    sqf = p3ps.tile([128, NT], F32, tag="mt3", name="mt3ssq0")
                    for k in range(16):
                        ht0 = p2.tile([128, NT], BF16, tag="ht0", name="ht0")
                        nc.sync.dma_start(ht0[:],
                                          io['hsT'][k * 128:(k + 1) * 128,
                                                    0:NT])
                        nc.vector.tensor_tensor(mt0[:, k, :], mt0[:, k, :],
                                                s3b0[:], Op.mult)
                        nc.vector.tensor_tensor(mt0[:, k, :], mt0[:, k, :],
                                                ht0[:], Op.add)
                        nc.sync.dma_start(
                            io['resid2T'][k * 128:(k + 1) * 128, 0:NT],
                            mt0[:, k, :])
                        sq0 = p2s.tile([128, NT], F32R, tag="sq0", name="sq0")
                        nc.scalar.activation(sq0[:], mt0[:, k, :], AF.Square)
                        mm(sqf[0:1, :], ones128[:], sq0[:],
                           start=(k == 0), stop=(k == 15))
                    slr0 = p2s.tile([1, NT], F32, tag="slr0", name="slr0")
                    nc.scalar.activation(slr0[:], sqf[0:1, :], AF.Ln,
                                         bias=eps1[:], scale=float(1.0 / H))
                    sr_0 = p2s.tile([1, NT], F32R, tag="sr_0", name="sr_0")
                    nc.scalar.activation(sr_0[:], slr0[:], AF.Exp, scale=-0.5)
                    sbp0 = p3ps.tile([128, NT], F32, tag="mt3", name="mt3sb0")
                    mm(sbp0[:], ones1[:], sr_0[:], start=True, stop=True)
                    sb0 = p2pre.tile([128, NT], BF16, tag="sb0", name="sb0")
                    nc.any.tensor_copy(sb0[:], sbp0[:])
                    mtn0 = p2pre.tile([128, 16, NT], BF16, tag="mtn0",
                                      name="mtn0")
                    for k in range(16):
                        nc.vector.scalar_tensor_tensor(
                            mtn0[:, k, :], mt0[:, k, :], C['ln2_c'][:, k:k + 1],
                            sb0[:], Op.mult, Op.mult)
                    nc.sync.dma_start(
                        scr['mtn0'].rearrange("(kt p) n -> p kt n", p=128),
                        mtn0[:])

        rows_a_es.close()

        # ================================= Phase 4: resid + ln2 + MLP + RS
        # Software-pipelined: tile j+1's resid/stats/mtn prep is emitted
        # between tile j's gate_up and down matmuls; tile 0's prep ran in
        # Phase 2 (bounced via scr['mtn0']).
        with tc.tile_pool(name="p4w", bufs=1) as p4w, \
             tc.tile_pool(name="p4", bufs=2) as p4, \
             tc.tile_pool(name="p4mt", bufs=1) as p4mt, \
             tc.tile_pool(name="p4row", bufs=1) as p4row, \
             tc.tile_pool(name="p4av", bufs=1) as p4av, \
             tc.tile_pool(name="p4ps_s", bufs=1, space="PSUM") as p4ps_s, \
             tc.tile_pool(name="p4ps_g", bufs=2, space="PSUM") as p4ps_g, \
             tc.tile_pool(name="p4ps_d", bufs=2, space="PSUM") as p4ps_d:
            # tile 0's mtn (precomputed in Phase 2) is fetched before the
            # weights so the first gate matmul isn't stuck behind 12MB of
            # weight DMA on the queue
            mtn_cur = p4.tile([128, 16, NT], BF16, tag="mtn", name="mtn")
            nc.sync.dma_start(mtn_cur[:], scr['mtn0']
                              .rearrange("(kt p) n -> p kt n", p=128))
            wg_t = p4w.tile([128, 16, FFr], BF16, tag="wg_t", name="wg_t")
            nc.sync.dma_start(wg_t[:],
                              io['w_gate'].rearrange("(kt p) m -> p kt m", p=128))
            wu_t = p4w.tile([128, 16, FFr], BF16, tag="wu_t", name="wu_t")
            nc.sync.dma_start(wu_t[:],
                              io['w_up'].rearrange("(kt p) m -> p kt m", p=128))
            wd_t = p4w.tile([128, 8, H], BF16, tag="wd_t", name="wd_t")
            nc.sync.dma_start(wd_t[:],
                              io['w_down'].rearrange("(kt p) m -> p kt m", p=128))

            def p4_prep_a(j):
                # DMA issue + s3 scale chain only: no heavy PE work, so this
                # can sit between gate and down without head-of-line blocking
                # the tensor queue.
                tok0 = j * NT
                mt = p4mt.tile([128, 16, NT], BF16, tag="mt", name="mt")
                nc.sync.dma_start(mt[:], scr['ar1_out8'][j]
                                  .rearrange("(kt p) n -> p kt n", p=128))
                ht_all = p4mt.tile([128, 16, NT], BF16, tag="ht_all",
                                   name="ht_all")
                nc.sync.dma_start(ht_all[:], io['hsT'][:, tok0:tok0 + NT]
                                  .rearrange("(kt p) n -> p kt n", p=128))
                ssq_t = p4row.tile([1, NT], F32, tag="ssq_t", name="ssq_t")
                nc.sync.dma_start(ssq_t[:], scr['ssq_out8'][j])
                ssq_l = p4row.tile([1, NT], F32, tag="ssq_l", name="ssq_l")
                nc.scalar.activation(ssq_l[:], ssq_t[:], AF.Ln,
                                     bias=eps1[:], scale=float(1.0 / DIN))
                s3_row = p4row.tile([1, NT], F32R, tag="s3row", name="s3row")
                nc.scalar.activation(s3_row[:], ssq_l[:], AF.Exp, scale=-0.5)
                s3b_ps = p4ps_s.tile([128, NT], F32, tag="bps", name="s3bps")
                mm(s3b_ps[:], ones1[:], s3_row[:], start=True, stop=True)
                s3b = p4.tile([128, NT], BF16, tag="s3b", name="s3b")
                nc.any.tensor_copy(s3b[:], s3b_ps[:])
                return mt, ht_all, s3b

            def p4_prep_b(j, pa):
                # the PE-heavy stats tail, emitted AFTER this tile's down
                # matmuls so those never stall behind it in the queue
                mt, ht_all, s3b = pa
                tok0 = j * NT
                ssq_ps = p4ps_s.tile([1, NT], F32, tag="ssq", name="ssq")
                for k in range(16):
                    nc.vector.tensor_tensor(mt[:, k, :], mt[:, k, :], s3b[:],
                                            Op.mult)
                    nc.vector.tensor_tensor(mt[:, k, :], mt[:, k, :],
                                            ht_all[:, k, :], Op.add)
                    nc.sync.dma_start(
                        io['resid2T'][k * 128:(k + 1) * 128, tok0:tok0 + NT],
                        mt[:, k, :])
                    sq = p4.tile([128, NT], F32R, tag="sq", name="sq")
                    nc.scalar.activation(sq[:], mt[:, k, :], AF.Square)
                    mm(ssq_ps[:], ones128[:], sq[:],
                       start=(k == 0), stop=(k == 15))
                sr0 = p4row.tile([1, NT], F32, tag="sr0", name="sr0")
                nc.scalar.activation(sr0[:], ssq_ps[:], AF.Ln,
                                     bias=eps1[:], scale=float(1.0 / H))
                s_row = p4row.tile([1, NT], F32R, tag="srow", name="srow")
                nc.scalar.activation(s_row[:], sr0[:], AF.Exp, scale=-0.5)
                sb_ps = p4ps_s.tile([128, NT], F32, tag="bps", name="sbps")
                mm(sb_ps[:], ones1[:], s_row[:], start=True, stop=True)
                sb = p4.tile([128, NT], BF16, tag="sb", name="sb")
                nc.any.tensor_copy(sb[:], sb_ps[:])
                mtn = p4.tile([128, 16, NT], BF16, tag="mtn", name="mtn")
                for k in range(16):
                    nc.vector.scalar_tensor_tensor(
                        mtn[:, k, :], mt[:, k, :], C['ln2_c'][:, k:k + 1],
                        sb[:], Op.mult, Op.mult)
                return mtn

            for nt in range(n8):
                tok0 = nt * NT
                # gate_up + silu*up (av kept in SBUF as down-proj k-tiles)
                av = p4av.tile([128, 8, NT], BF16, tag="av", name="av")
                for mi in range(8):
                    gp = p4ps_g.tile([128, NT], F32, tag="gp", name="gp")
                    up = p4ps_g.tile([128, NT], F32, tag="up", name="up")
                    for k in range(16):
                        mm(gp[:], wg_t[:, k, mi * 128:(mi + 1) * 128],
                           mtn_cur[:, k, :], start=(k == 0), stop=(k == 15))
                    for k in range(16):
                        mm(up[:], wu_t[:, k, mi * 128:(mi + 1) * 128],
                           mtn_cur[:, k, :], start=(k == 0), stop=(k == 15))
                    sg = p4.tile([128, NT], BF16, tag="sg", name="sg")
                    silu(sg[:], gp[:], pool=p4, tag="sgsig")
                    nc.vector.tensor_tensor(av[:, mi, :], sg[:], up[:], Op.mult)
                # next tile's prep DMA + scale chain lands between the gate
                # and down matmuls; its PE-heavy stats tail is emitted after
                # down so the down matmuls never stall behind it
                pa_next = p4_prep_a(nt + 1) if nt + 1 < n8 else None
                # down proj -> ReduceScatter chunk (host concats slices).
                # The LAST tile is split into four token-quarters so the
                # end-of-kernel collective tail shrinks to ~one quarter RS.
                if nt < n8 - 1:
                    for mo in range(16):
                        ps = p4ps_d.tile([128, NT], F32, tag="dps", name="dps")
                        for k in range(8):
                            mm(ps[:], wd_t[:, k, mo * 128:(mo + 1) * 128],
                               av[:, k, :], start=(k == 0), stop=(k == 7))
                        ot = p4.tile([128, NT], BF16, tag="ot4", name="ot4")
                        nc.any.tensor_copy(ot[:], ps[:])
                        nc.sync.dma_start(
                            scr['rs2_in8'][nt][mo * 128:(mo + 1) * 128, :],
                            ot[:])
                    if world > 1:
                        nc.gpsimd.collective_compute(
                            "ReduceScatter", Op.add,
                            replica_groups=[list(range(world))],
                            ins=[scr['rs2_in8'][nt]],
                            outs=[scr['rs2_out8'][nt]])
                    else:
                        nc.sync.dma_start(scr['rs2_out8'][nt],
                                          scr['rs2_in8'][nt][0:H // world, :])
                    nc.sync.dma_start(io['out1T'][:, tok0:tok0 + NT],
                                      scr['rs2_out8'][nt])
                else:
                    NH4 = NT // 4
                    for qr in range(4):
                        c0 = qr * NH4
                        for mo in range(16):
                            ps = p4ps_d.tile([128, NH4], F32, tag="dps",
                                             name="dps")
                            for k in range(8):
                                mm(ps[:], wd_t[:, k, mo * 128:(mo + 1) * 128],
                                   av[:, k, c0:c0 + NH4],
                                   start=(k == 0), stop=(k == 7))
                            ot = p4.tile([128, NH4], BF16, tag="ot4",
                                         name="ot4")
                            nc.any.tensor_copy(ot[:], ps[:])
                            nc.sync.dma_start(
                                scr['rs2h_in'][qr][mo * 128:(mo + 1) * 128,
                                                   :], ot[:])
                        if world > 1:
                            nc.gpsimd.collective_compute(
                                "ReduceScatter", Op.add,
                                replica_groups=[list(range(world))],
                                ins=[scr['rs2h_in'][qr]],
                                outs=[scr['rs2h_out'][qr]])
                        else:
                            nc.sync.dma_start(
                                scr['rs2h_out'][qr],
                                scr['rs2h_in'][qr][0:H // world, :])
                        nc.sync.dma_start(
                            io['out1T'][:, tok0 + c0:tok0 + c0 + NH4],
                            scr['rs2h_out'][qr])
                mtn_cur = p4_prep_b(nt + 1, pa_next) if pa_next else None


# ================================================================ entry point
def kernel(**inputs):
    from concourse import bass_utils

    nc = build(world=TP, debug=False)
    in_maps = [shard_core_inputs(inputs, r) for r in range(TP)]
    res = bass_utils.run_bass_kernel_spmd(nc, in_maps, core_ids=list(range(TP)))
    out1T = np.concatenate(
        [np.asarray(res.results[r]['out1T'], dtype=np.float32)
         for r in range(TP)], axis=0)                # [H, T] feature-major
    out1 = np.ascontiguousarray(out1T.T).reshape(B, L, H)
    resid2 = np.ascontiguousarray(
        np.asarray(res.results[0]['resid2T'], dtype=np.float32).T
    ).reshape(B, L, H)
    return out1, resid2


if __name__ == '__main__':
    nc = build(world=1)
    print("built ok")



# revision 12
# speedup vs baseline: 1.0018x; 1.0018x over previous
"""Trainium2 Bass kernel for nn_BambaMixerDecoderLayer_84696755077458.

Tensor-parallel over 8 NeuronCores (vLLM-style), v2 (bf16):
  - in_proj / gate_up column-sharded, out_proj / down row-sharded
  - heads + conv channels sharded with d_inner; B/C conv streams replicated
  - SSM scan via chunked SSD (Mamba2): intra-chunk matmuls + small
    cross-chunk state recurrence.
  - bf16 weights/activations for all large GEMMs, scratch and collectives;
    fp32 for stats, decay rows and the SSD state.
  - Single merged in_proj pass; MLP (gate_up+down) fused in one pass.
  - Collectives chunked 8x along tokens and issued inline so they overlap
    with compute (no global barrier between SSD and MLP phases).
Everything on-device is feature-major ([feature, token]); host does layout
transforms (transpose / shard / concat) only.

Self-contained: hardcodes all shapes; needs only /opt/trn_rl_repo on sys.path.
"""
import sys
from contextlib import ExitStack

if '/opt/trn_rl_repo' not in sys.path:
    sys.path.insert(0, '/opt/trn_rl_repo')

import numpy as np

# ---------------------------------------------------------------- constants
H = 2048          # hidden
DIN = 4096        # mamba intermediate
DS = 128          # ssm state
DCONV = 4
NH = 64
HD = 64
FF = 8192
EPS = 1e-5
B, L = 2, 2048
T = B * L                         # 4096 tokens
CONV_DIM = DIN + 2 * DS           # 4352
D_IN_PROJ = 2 * DIN + 2 * DS + NH  # 8512

TP = 8
NHr = NH // TP                    # 8 heads / core
DINr = DIN // TP                  # 512
FFr = FF // TP                    # 1024
CONVr = DINr + 2 * DS             # 768 conv channels / core
MPROJ = DINr + CONVr + NHr        # 1288 in_proj cols / core

Q = 128                           # SSD chunk
NT = 512                          # token tile (also the collective chunk)
NEG = -3.0e38
SIM_SILU = False   # True: emit sigmoid+mul instead of Silu (CoreSim support)


def _f32(x):
    return np.ascontiguousarray(np.asarray(x, dtype=np.float32))


def _bf16(x):
    import ml_dtypes
    return np.ascontiguousarray(
        np.asarray(x, dtype=np.float32).astype(ml_dtypes.bfloat16))


# ================================================================ host prep
def host_constants():
    import ml_dtypes
    identb = np.eye(128, dtype=ml_dtypes.bfloat16)
    i8 = np.eye(8, dtype=np.float32)
    sel8 = np.zeros((8, 8 * 128), np.float32)
    for h in range(8):
        sel8[h, h * 128:(h + 1) * 128] = 1.0
    # negsel4[4g+i, g*512 + i*128 + q'] = -1  (col term for group-of-4 segsum)
    negsel4 = np.zeros((8, 1024), np.float32)
    for h in range(8):
        negsel4[h, h * 128:(h + 1) * 128] = -1.0
    ones8 = np.ones((8, 128), np.float32)
    ones1 = np.ones((1, 128), np.float32)
    ones128 = np.ones((128, 1), np.float32)
    tri = np.where(np.arange(Q)[:, None] > np.arange(Q)[None, :], NEG, 0.0)
    trimask4 = np.concatenate([tri] * 4, axis=1).astype(ml_dtypes.bfloat16)
    return dict(c_identb=identb, c_i8=i8, c_sel8=sel8,
                c_sel8b=sel8.astype(ml_dtypes.bfloat16), c_negsel4=negsel4,
                c_ones8=ones8, c_ones1=ones1, c_ones128=ones128,
                c_trimask4=trimask4)


def shard_core_inputs(inputs, r):
    """Build the per-core input map (all feature-major)."""
    w_in = _f32(inputs['w_in'])
    zs = slice(DINr * r, DINr * (r + 1))
    xs = slice(DIN + DINr * r, DIN + DINr * (r + 1))
    bs = slice(2 * DIN, 2 * DIN + DS)
    cs = slice(2 * DIN + DS, 2 * DIN + 2 * DS)
    dts = slice(2 * DIN + 2 * DS + NHr * r, 2 * DIN + 2 * DS + NHr * (r + 1))
    w_in_r = np.concatenate(
        [w_in[:, zs], w_in[:, xs], w_in[:, bs], w_in[:, cs], w_in[:, dts]], axis=1)

    conv_w = _f32(inputs['conv_w'])
    conv_w_r = np.concatenate([conv_w[DINr * r:DINr * (r + 1)], conv_w[DIN:]], axis=0)
    conv_b = _f32(inputs['conv_b'])
    conv_b_r = np.concatenate([conv_b[DINr * r:DINr * (r + 1)], conv_b[DIN:]], axis=0)

    hs = _f32(inputs['hidden_states'])
    hs = hs.reshape(-1, H)

    A_r = _f32(inputs['A_log'])[NHr * r:NHr * (r + 1)]
    dtb_r = _f32(inputs['dt_bias'])[NHr * r:NHr * (r + 1)]
    D_r = _f32(inputs['D_ssm'])[NHr * r:NHr * (r + 1)]

    m = dict(host_constants())
    m['hsT'] = _bf16(hs.T)                                      # [2048, T]
    m['w_in'] = _bf16(w_in_r)                                   # [2048, 1288]
    # per-k-tile column form of per-feature vectors: [128, n_tiles]
    m['ln1_c'] = np.ascontiguousarray(_f32(inputs['ln1_w']).reshape(16, 128).T)
    m['ln2_c'] = np.ascontiguousarray(_f32(inputs['ln2_w']).reshape(16, 128).T)
    m['normw_c'] = np.ascontiguousarray(
        _f32(inputs['norm_w'])[DINr * r:DINr * (r + 1)].reshape(4, 128).T)
    m['dssm_c'] = np.ascontiguousarray(
        np.repeat(D_r, HD).reshape(4, 128).T)                   # [128, 4]
    # conv weights: [128, 6*4] with [p, pt*4+d]
    m['conv_w'] = np.ascontiguousarray(
        conv_w_r.reshape(6, 128, DCONV).transpose(1, 0, 2).reshape(128, 6 * DCONV))
    m['conv_b'] = np.ascontiguousarray(conv_b_r.reshape(6, 128).T)  # [128, 6]
    m['a_col'] = np.ascontiguousarray((-np.exp(A_r))[:, None])   # [8,1]
    m['dtb_col'] = np.ascontiguousarray(dtb_r[:, None])          # [8,1]
    m['w_out'] = _bf16(_f32(inputs['w_out'])[DINr * r:DINr * (r + 1)])
    wgu = _f32(inputs['w_gate_up'])
    m['w_gate'] = _bf16(wgu[:, FFr * r:FFr * (r + 1)])
    m['w_up'] = _bf16(wgu[:, FF + FFr * r:FF + FFr * (r + 1)])
    m['w_down'] = _bf16(_f32(inputs['w_down'])[FFr * r:FFr * (r + 1)])
    return m


# ================================================================ the kernel
def build(world=TP, debug=False, T_=T):
    import concourse.mybir as mybir
    import concourse.tile as tile
    from concourse import bacc
    from concourse.alu_op_type import AluOpType as Op

    AF = mybir.ActivationFunctionType
    F32 = mybir.dt.float32
    BF16 = mybir.dt.bfloat16

    nc = bacc.Bacc("TRN2", target_bir_lowering=False, debug=False,
                   num_devices=world)

    F32R = mybir.dt.float32r
    n8 = T_ // NT

    def din(name, shape, dt):
        return nc.dram_tensor(name, list(shape), dt, kind="ExternalInput").ap()

    BIN = {'hsT', 'w_in', 'w_out', 'w_gate', 'w_up', 'w_down', 'c_identb',
           'c_trimask4', 'c_sel8b'}
    RIN = {'c_i8', 'c_sel8', 'c_negsel4', 'c_ones8', 'c_ones1', 'c_ones128'}
    io = {}
    for name, shape in [
            ('hsT', (H, T_)), ('w_in', (H, MPROJ)),
            ('ln1_c', (128, 16)), ('ln2_c', (128, 16)),
            ('normw_c', (128, 4)), ('dssm_c', (128, 4)),
            ('conv_w', (128, 24)), ('conv_b', (128, 6)),
            ('a_col', (8, 1)), ('dtb_col', (8, 1)),
            ('w_out', (DINr, H)), ('w_gate', (H, FFr)), ('w_up', (H, FFr)),
            ('w_down', (FFr, H)),
            ('c_identb', (128, 128)), ('c_i8', (8, 8)), ('c_sel8', (8, 1024)),
            ('c_sel8b', (8, 1024)), ('c_negsel4', (8, 1024)), ('c_ones8', (8, 128)),
            ('c_ones1', (1, 128)), ('c_ones128', (128, 1)),
            ('c_trimask4', (128, 512))]:
        dt = BF16 if name in BIN else (F32R if name in RIN else F32)
        io[name] = din(name, shape, dt)

    io['out1T'] = nc.dram_tensor("out1T", [H // world, T_], BF16,
                                 kind="ExternalOutput").ap()
    io['resid2T'] = nc.dram_tensor("resid2T", [H, T_], BF16,
                                   kind="ExternalOutput").ap()

    skind = "ExternalOutput" if debug else "Internal"
    scr = {}
    scr['z'] = nc.dram_tensor("z_s", [DINr, T_], BF16, kind=skind).ap()
    scr['x'] = nc.dram_tensor("x_s", [DINr, T_], BF16, kind=skind).ap()
    scr['b'] = nc.dram_tensor("b_s", [DS, T_], BF16, kind=skind).ap()
    scr['c'] = nc.dram_tensor("c_s", [DS, T_], BF16, kind=skind).ap()
    scr['ar1_in8'] = [
        nc.dram_tensor(f"ar1_in{q}", [H, NT], BF16, kind="Internal").ap()
        for q in range(n8)]
    scr['ar1_out8'] = [
        nc.dram_tensor(f"ar1_out{q}", [H, NT], BF16, kind="Internal",
                       addr_space="Shared").ap() for q in range(n8)]
    scr['ssq_in8'] = [
        nc.dram_tensor(f"ssq_in{q}", [1, NT], F32, kind="Internal").ap()
        for q in range(n8)]
    scr['ssq_out8'] = [
        nc.dram_tensor(f"ssq_out{q}", [1, NT], F32, kind="Internal",
                       addr_space="Shared").ap() for q in range(n8)]
    scr['rs2_in8'] = [
        nc.dram_tensor(f"rs2_in{q}", [H, NT], BF16, kind="Internal").ap()
        for q in range(n8)]
    scr['rs2_out8'] = [
        nc.dram_tensor(f"rs2_out{q}", [H // world, NT], BF16,
                       kind="Internal").ap() for q in range(n8)]
    scr['mtn0'] = nc.dram_tensor("mtn0_s", [H, NT], BF16, kind="Internal").ap()
    scr['rs2h_in'] = [
        nc.dram_tensor(f"rs2h_in{q}", [H, NT // 4], BF16,
                       kind="Internal").ap() for q in range(4)]
    scr['rs2h_out'] = [
        nc.dram_tensor(f"rs2h_out{q}", [H // world, NT // 4], BF16,
                       kind="Internal").ap() for q in range(4)]

    with tile.TileContext(nc) as tc:
        _body(tc, io, scr, world, debug, mybir, tile, Op, AF, F32, T_)

    nc.compile()
    return nc


def _body(tc, io, scr, world, debug, mybir, tile, Op, AF, F32, T_):
    nc = tc.nc
    F32R = mybir.dt.float32r
    BF16 = mybir.dt.bfloat16
    n8 = T_ // NT
    NCHUNK = T_ // Q
    CPS = (T_ // B) // Q          # chunks per sequence

    def mm(out, lhsT, rhs, start, stop, skip=False):
        if lhsT.dtype == F32:
            lhsT = lhsT.bitcast(F32R)
        if rhs.dtype == F32:
            rhs = rhs.bitcast(F32R)
        nc.tensor.matmul(out, lhsT, rhs, start=start, stop=stop,
                         skip_group_check=skip)

    def silu(out_ap, in_ap, bias=0.0, pool=None, tag="silu_tmp"):
        if SIM_SILU:
            tmp = pool.tile(list(out_ap.shape), F32, tag=tag, name=tag)
            nc.scalar.activation(tmp[:], in_ap, AF.Sigmoid, bias=bias, scale=1.0)
            if isinstance(bias, float) and bias == 0.0:
                nc.vector.tensor_tensor(out_ap, in_ap, tmp[:], Op.mult)
            else:
                raise NotImplementedError("SIM_SILU with bias AP")
        else:
            nc.scalar.activation(out_ap, in_ap, AF.Silu, bias=bias, scale=1.0)

    def allreduce(in_ap, out_ap):
        if world > 1:
            nc.gpsimd.collective_compute(
                "AllReduce", Op.add, replica_groups=[list(range(world))],
                ins=[in_ap], outs=[out_ap])
        else:
            nc.sync.dma_start(out_ap, in_ap)

    with ExitStack() as ES:
        cpool = ES.enter_context(tc.tile_pool(name="consts", bufs=1))

        # -------------------------------------------------------- constants
        C = {}
        RT = {'c_i8', 'c_sel8', 'c_negsel4', 'c_ones8', 'c_ones1',
              'c_ones128'}
        BT = {'c_identb', 'c_trimask4', 'c_sel8b'}
        for nm, shape in [('c_identb', (128, 128)), ('c_i8', (8, 8)),
                          ('c_sel8', (8, 1024)), ('c_sel8b', (8, 1024)),
                          ('c_negsel4', (8, 1024)),
                          ('c_ones8', (8, 128)), ('c_ones1', (1, 128)),
                          ('c_ones128', (128, 1)), ('c_trimask4', (128, 512)),
                          ('ln1_c', (128, 16)), ('ln2_c', (128, 16)),
                          ('normw_c', (128, 4)), ('dssm_c', (128, 4)),
                          ('conv_w', (128, 24)), ('conv_b', (128, 6)),
                          ('a_col', (8, 1)), ('dtb_col', (8, 1))]:
            dt = BF16 if nm in BT else (F32R if nm in RT else F32)
            t = cpool.tile(list(shape), dt, tag=nm)
            nc.sync.dma_start(t[:], io[nm])
            C[nm] = t
        identb, i8 = C['c_identb'], C['c_i8']
        sel8, negsel4 = C['c_sel8'], C['c_negsel4']
        sel8b = C['c_sel8b']
        ones8, ones1, ones128 = C['c_ones8'], C['c_ones1'], C['c_ones128']
        trimask4 = C['c_trimask4']

        eps1 = cpool.tile([1, 1], F32, tag="eps1", name="eps1")
        nc.vector.memset(eps1[:], float(EPS))

        # ======================================================== Phase 1
        # merged single pass over hsT: ln1 stats + z + dt + xBC + conv
        rows_a_es = ExitStack()
        rows_a = rows_a_es.enter_context(tc.tile_pool(name="rows_a", bufs=1))
        dt_rows = rows_a.tile([8, T_], F32R, tag="dt_rows", name="dt_rows")
        lA_rows = rows_a.tile([8, T_], F32R, tag="lA_rows", name="lA_rows")
        ssq_yz_row = rows_a.tile([1, T_], F32, tag="ssq_yz", name="ssq_yz")

        with tc.tile_pool(name="p1w", bufs=1) as p1w, \
             tc.tile_pool(name="p1", bufs=2) as p1, \
             tc.tile_pool(name="convp", bufs=2) as convp, \
             tc.tile_pool(name="p1ps_s", bufs=1, space="PSUM") as p1ps_s, \
             tc.tile_pool(name="p1ps_m", bufs=2, space="PSUM") as p1ps_m:

            # first token tile is prefetched BEFORE the weights so the ln1
            # stats matmuls warm up the PE while w_in streams in
            hst0 = p1.tile([128, 16, NT], BF16, tag="hst", name="hst")
            nc.sync.dma_start(hst0[:, 0:8, :], io['hsT'][0:8 * 128, 0:NT]
                              .rearrange("(kt p) n -> p kt n", p=128))
            nc.sync.dma_start(hst0[:, 8:16, :], io['hsT'][8 * 128:H, 0:NT]
                              .rearrange("(kt p) n -> p kt n", p=128))
            # all in_proj columns per core: [z | xBC | dt] = 1288
            w1 = p1w.tile([128, 16, MPROJ], BF16, tag="w1", name="w1")
            nc.sync.dma_start(
                w1[:], io['w_in'].rearrange("(kt p) m -> p kt m", p=128))
            for k in range(16):
                nc.vector.tensor_scalar_mul(w1[:, k, :], w1[:, k, :],
                                            C['ln1_c'][:, k:k + 1])

            halo_prev = None
            for nt in range(n8):
                tok0 = nt * NT
                seq_start = (tok0 % (T_ // B)) == 0
                if nt == 0:
                    hst = hst0
                else:
                    hst = p1.tile([128, 16, NT], BF16, tag="hst", name="hst")
                    nc.sync.dma_start(hst[:], io['hsT'][:, tok0:tok0 + NT]
                                      .rearrange("(kt p) n -> p kt n", p=128))
                # ln1 stats (ACT squares; matmuls never wait on these)
                ssq_ps = p1ps_s.tile([1, NT], F32, tag="ssq", name="ssq")
                for k in range(16):
                    sq = p1.tile([128, NT], F32R, tag="sq", name="sq")
                    nc.scalar.activation(sq[:], hst[:, k, :], AF.Square)
                    mm(ssq_ps[:], ones128[:], sq[:],
                       start=(k == 0), stop=(k == 15))
                sr0 = p1.tile([1, NT], F32, tag="sr0", name="sr0", bufs=1)
                nc.scalar.activation(sr0[:], ssq_ps[:], AF.Ln,
                                     bias=eps1[:], scale=float(1.0 / H))
                s_row = p1.tile([1, NT], F32R, tag="s_row", name="s_row",
                                bufs=1)
                nc.scalar.activation(s_row[:], sr0[:], AF.Exp, scale=-0.5)
                sb_ps = p1ps_s.tile([128, NT], F32, tag="sbps", name="sbps")
                mm(sb_ps[:], ones1[:], s_row[:], start=True, stop=True)
                sb = p1.tile([128, NT], F32, tag="sb", name="sb")
                nc.any.tensor_copy(sb[:], sb_ps[:])
                # z m-tiles: matmul on RAW hidden, scale on the way out
                for mi in range(4):
                    ps = p1ps_m.tile([128, NT], F32, tag="mt", name="mt")
                    for k in range(16):
                        mm(ps[:], w1[:, k, mi * 128:(mi + 1) * 128],
                           hst[:, k, :], start=(k == 0), stop=(k == 15))
                    zt = p1.tile([128, NT], BF16, tag="z", name="z")
                    nc.vector.tensor_tensor(zt[:], ps[:], sb[:], Op.mult)
                    nc.sync.dma_start(
                        scr['z'][mi * 128:(mi + 1) * 128, tok0:tok0 + NT], zt[:])
                # dt m-tile (8 wide)
                dtp = p1ps_s.tile([8, NT], F32, tag="mtdt", name="mtdt")
                for k in range(16):
                    mm(dtp[:], w1[:, k, DINr + CONVr:MPROJ], hst[:, k, :],
                       start=(k == 0), stop=(k == 15))
                dt_raw = p1.tile([8, NT], F32, tag="dtraw", name="dtraw",
                                 bufs=1)
                nc.vector.tensor_tensor(dt_raw[:], dtp[:], sb[:8, :], Op.mult)
                e8 = p1.tile([8, NT], F32, tag="e8", name="e8", bufs=1)
                nc.scalar.activation(e8[:], dt_raw[:], AF.Exp,
                                     bias=C['dtb_col'][:], scale=1.0)
                e8p = p1.tile([8, NT], F32, tag="e8p", name="e8p", bufs=1)
                nc.vector.tensor_scalar_add(e8p[:], e8[:], 1.0)
                nc.scalar.activation(dt_rows[:, tok0:tok0 + NT], e8p[:], AF.Ln)
                logda = p1.tile([8, NT], F32, tag="logda", name="logda",
                                bufs=1)
                nc.vector.tensor_scalar_mul(logda[:], dt_rows[:, tok0:tok0 + NT],
                                            C['a_col'][:])
                for c in range(NT // Q):
                    nc.vector.tensor_tensor_scan(
                        lA_rows[:, tok0 + c * Q:tok0 + (c + 1) * Q],
                        ones8[:, :Q].bitcast(F32), logda[:, c * Q:(c + 1) * Q],
                        0.0, Op.mult, Op.add)
                # xBC m-tiles + causal conv
                halo = [convp.tile([128, NT + 3], BF16, tag=f"halo{pt}",
                                   name=f"halo{pt}") for pt in range(6)]
                for pt in range(6):
                    ps = p1ps_m.tile([128, NT], F32, tag="mt", name="mt")
                    for k in range(16):
                        mm(ps[:], w1[:, k, DINr + pt * 128:DINr + (pt + 1) * 128],
                           hst[:, k, :], start=(k == 0), stop=(k == 15))
                    nc.vector.tensor_tensor(halo[pt][:, 3:3 + NT], ps[:], sb[:],
                                            Op.mult)
                for pt in range(6):
                    if seq_start:
                        nc.vector.memset(halo[pt][:, 0:3], 0.0)
                    else:
                        nc.vector.tensor_copy(halo[pt][:, 0:3],
                                              halo_prev[pt][:, NT:NT + 3])
                    acc = convp.tile([128, NT], BF16, tag="cacc", name="cacc")
                    nc.vector.tensor_scalar_mul(
                        acc[:], halo[pt][:, 0:NT],
                        C['conv_w'][:, pt * 4:pt * 4 + 1])
                    for d in range(1, 4):
                        nc.vector.scalar_tensor_tensor(
                            acc[:], halo[pt][:, d:d + NT],
                            C['conv_w'][:, pt * 4 + d:pt * 4 + d + 1],
                            acc[:], Op.mult, Op.add)
                    cact = convp.tile([128, NT], BF16, tag="cact", name="cact")
                    if SIM_SILU:
                        nc.vector.tensor_scalar_add(acc[:], acc[:],
                                                    C['conv_b'][:, pt:pt + 1])
                        silu(cact[:], acc[:], pool=convp, tag="cvsig")
                    else:
                        nc.scalar.activation(cact[:], acc[:], AF.Silu,
                                             bias=C['conv_b'][:, pt:pt + 1],
                                             scale=1.0)
                    if pt < 4:
                        nc.sync.dma_start(
                            scr['x'][pt * 128:(pt + 1) * 128, tok0:tok0 + NT],
                            cact[:])
                    elif pt == 4:
                        nc.sync.dma_start(scr['b'][:, tok0:tok0 + NT], cact[:])
                    else:
                        nc.sync.dma_start(scr['c'][:, tok0:tok0 + NT], cact[:])
                halo_prev = halo

        # ============================================ Phase 2: SSD + gated
        # norm + out_proj, fused per token-tile. out_proj runs on UNSCALED
        # yz — the rms scale s3 commutes through the matmul and the
        # AllReduce, and is applied in Phase 4. AR chunks issued inline.
        with tc.tile_pool(name="p2", bufs=3) as p2, \
             tc.tile_pool(name="p2s", bufs=2) as p2s, \
             tc.tile_pool(name="state", bufs=1) as spool, \
             tc.tile_pool(name="p2pre", bufs=1) as p2pre, \
             tc.tile_pool(name="p3f", bufs=2) as p3f, \
             tc.tile_pool(name="p3w", bufs=1) as p3w, \
             tc.tile_pool(name="p2ps1", bufs=1, space="PSUM") as p2ps1, \
             tc.tile_pool(name="p2ps2", bufs=1, space="PSUM") as p2ps2, \
             tc.tile_pool(name="p3ps", bufs=2, space="PSUM") as p3ps:

            w_out_t = p3w.tile([128, 4, H], BF16, tag="w_out_t", name="w_out_t")
            nc.sync.dma_start(w_out_t[:],
                              io['w_out'].rearrange("(kt p) m -> p kt m", p=128))
            for k in range(4):
                nc.vector.tensor_scalar_mul(w_out_t[:, k, :], w_out_t[:, k, :],
                                            C['normw_c'][:, k:k + 1])

            S_all = spool.tile([128, NHr * HD], F32R, tag="S_all", name="S_all")
            nc.vector.memset(S_all[:].bitcast(F32), 0.0)
            # bf16 shadow of the state for the y_inter matmul (PE cannot mix
            # f32r weights with bf16 moving); fp32 master stays exact
            S_b = spool.tile([128, NHr * HD], BF16, tag="S_b", name="S_b")
            nc.vector.memset(S_b[:], 0.0)

            for nt in range(n8):
                y_sb = p3f.tile([128, 4, NT], BF16, tag="ysb", name="ysb")
                for cc_ in range(NT // Q):
                    ch = nt * (NT // Q) + cc_
                    t0 = ch * Q
                    xf = p2.tile([128, 4, Q], BF16, tag="xf", name="xf")
                    nc.sync.dma_start(xf[:], scr['x'][:, t0:t0 + Q]
                                      .rearrange("(pt p) n -> p pt n", p=128))
                    bf = p2.tile([128, Q], BF16, tag="bf", name="bf")
                    nc.sync.dma_start(bf[:], scr['b'][:, t0:t0 + Q])
                    cf = p2.tile([128, Q], BF16, tag="cf", name="cf")
                    nc.sync.dma_start(cf[:], scr['c'][:, t0:t0 + Q])

                    lrow = lA_rows[:, t0:t0 + Q]
                    dtrow = dt_rows[:, t0:t0 + Q]

                    # exp(lA) rows: bf16 copy feeds the equad broadcasts;
                    # last column stays fp32 (chunk decay for the state).
                    explb = p2s.tile([8, Q], BF16, tag="explb", name="explb")
                    nc.scalar.activation(explb[:], lrow, AF.Exp)
                    expl_l = p2s.tile([8, 1], F32, tag="expl_l", name="expl_l")
                    nc.scalar.activation(expl_l[:], lrow[:, Q - 1:Q], AF.Exp)
                    ddr0 = p2s.tile([8, Q], F32, tag="ddr0", name="ddr0")
                    nc.vector.tensor_scalar(ddr0[:], lrow, -1.0,
                                            lrow[:, Q - 1:Q].bitcast(F32),
                                            Op.mult, Op.add)
                    dd_rows = p2s.tile([8, Q], F32R, tag="ddrows", name="ddrows")
                    nc.scalar.activation(dd_rows[:], ddr0[:], AF.Exp)
                    nc.vector.tensor_tensor(dd_rows[:], dd_rows[:], dtrow,
                                            Op.mult)
                    dg = p2s.tile([8, 8], F32R, tag="dg", name="dg")
                    nc.vector.tensor_scalar_mul(dg[:], i8[:], expl_l[:])

                    misc = p2ps1.tile([128, 160], F32, tag="misc", name="misc")
                    g_ps = misc[:, 0:128]
                    ddcol_ps = misc[:, 128:136]
                    decay_ps = misc[:, 136:144]
                    dtcol_ps = misc[:, 144:152]

                    mm(g_ps, bf[:], cf[:], start=True, stop=True)
                    mm(ddcol_ps, dd_rows[:], i8[:], start=True, stop=True)
                    mm(decay_ps, ones8[:], dg[:], start=True, stop=True)
                    mm(dtcol_ps, dtrow, i8[:], start=True, stop=True)
                    # late-read scalars leave PSUM early so misc can
                    # single-buffer without serializing chunks
                    g_sb = p2s.tile([128, 128], BF16, tag="g_sb", name="g_sb")
                    nc.any.tensor_copy(g_sb[:], g_ps)
                    dsc = p2s.tile([128, 16], F32, tag="dsc", name="dsc")
                    nc.any.tensor_copy(dsc[:], misc[:, 128:144])
                    ddcol_sb = dsc[:, 0:8]
                    decay_sb = dsc[:, 8:16]
                    dtc_b = p2s.tile([128, 8], BF16, tag="dtc_b", name="dtc_b")
                    nc.any.tensor_copy(dtc_b[:], dtcol_ps)

                    tps = p2ps1.tile([128, 5, 128], BF16, tag="xtm", name="xtm")
                    nc.tensor.transpose(tps[:, 4, :], bf[:], identb[:])
                    btm = p2s.tile([128, Q], BF16, tag="btm", name="btm")
                    nc.any.tensor_copy(btm[:], tps[:, 4, :])

                    for pt in range(4):
                        nc.tensor.transpose(tps[:, pt, :],
                                            xf[:, pt, :], identb[:])
                    xtm = p2s.tile([128, NHr, HD], BF16, tag="xtm_sb",
                                   name="xtm_sb")
                    nc.any.tensor_copy(
                        xtm[:], tps[:, 0:4, :].rearrange(
                            "p f (h d) -> p (f h) d", d=HD))
                    xw = p2s.tile([128, NHr, HD], BF16, tag="xw", name="xw")
                    nc.vector.tensor_tensor(
                        xw[:], xtm[:],
                        ddcol_sb[:, :, None].broadcast_to([128, NHr, HD]),
                        Op.mult)

                    # group-of-4 segsum: per-head broadcasts + batched col
                    # and tri-mask accumulated in PSUM fp32 (exact diagonal).
                    # Both groups' matmuls are emitted back-to-back (seg is
                    # double-buffered) so the ACT/DVE consumer chain of group
                    # 0 hides under group 1's PE burst.
                    segs = []
                    for g in range(2):
                        seg = p2ps2.tile([128, 2, 4, 128], F32, tag="seg",
                                         name="seg")
                        segs.append(seg)
                        # PSUM zero-region semantics: start=True re-marks the
                        # whole 2KB bank pending-zero, so exactly ONE start per
                        # bank; later first-touch writes overwrite via pending
                        # bits and full-width writes accumulate.
                        for i in range(4):
                            h = 4 * g + i
                            mm(seg[:, 0, i, :],
                               sel8[:, h * 128:(h + 1) * 128], lrow,
                               start=(i == 0), stop=False, skip=True)
                        mm(seg[:, 0], lrow,
                           negsel4[:, g * 512:(g + 1) * 512],
                           start=False, stop=False, skip=True)
                        mm(seg[:, 0], identb[:], trimask4[:],
                           start=False, stop=True, skip=True)
                        for i in range(4):
                            h = 4 * g + i
                            mm(seg[:, 1, i, :],
                               sel8b[:, h * 128:(h + 1) * 128], explb[:],
                               start=(i == 0), stop=(i == 3), skip=True)
                    for g in range(2):
                        seg = segs[g]
                        w0 = p2s.tile([128, 4, 128], BF16, tag="w0", name="w0")
                        nc.scalar.activation(w0[:], seg[:, 0], AF.Exp)
                        wt = p2s.tile([128, 4, 128], BF16, tag="wt", name="wt")
                        nc.vector.tensor_tensor(
                            wt[:], w0[:],
                            dtc_b[:, 4 * g:4 * g + 4, None]
                            .broadcast_to([128, 4, 128]), Op.mult)
                        nc.vector.tensor_tensor(
                            wt[:], wt[:],
                            g_sb[:, None, :].broadcast_to([128, 4, 128]),
                            Op.mult)
                        ce = p2s.tile([128, 4, 128], BF16, tag="ce", name="ce")
                        nc.vector.tensor_tensor(
                            ce[:], seg[:, 1],
                            cf[:, None, :].broadcast_to([128, 4, 128]),
                            Op.mult)
                        # y accumulates into the (already consumed) dquad bank
                        # of this group's seg tile: saves a PSUM bank so seg
                        # can double-buffer.
                        # head 4g+2j   -> partitions 0:64,  col block j
                        # head 4g+2j+1 -> partitions 64:128, col block j
                        for i in range(4):
                            h = 4 * g + i
                            j, half = i // 2, i % 2
                            ysl = seg[64 * half:64 * (half + 1), 0, j, :]
                            mm(ysl, xtm[:, h, :], wt[:, i, :],
                               start=True, stop=False, skip=True)
                            mm(ysl, S_b[:, h * HD:(h + 1) * HD],
                               ce[:, i, :], start=False, stop=True, skip=True)
                        for j in range(2):
                            pt = 2 * g + j
                            nc.vector.scalar_tensor_tensor(
                                y_sb[:, pt, cc_ * Q:(cc_ + 1) * Q],
                                xf[:, pt, :], C['dssm_c'][:, pt:pt + 1],
                                seg[:, 0, j, :], Op.mult, Op.add)

                    tp_ps = p2ps1.tile([128, 512], F32, tag="tp", name="tp")
                    mm(tp_ps[:], btm[:], xw[:], start=True, stop=True)
                    S3 = S_all[:].rearrange("p (h d) -> p h d", d=HD)
                    nc.vector.tensor_tensor(
                        S3, S3,
                        decay_sb[:, :, None].broadcast_to([128, NHr, HD])
                        .bitcast(F32), Op.mult)
                    nc.vector.tensor_tensor(
                        S3, S3,
                        tp_ps[:].rearrange("p (h d) -> p h d", d=HD),
                        Op.add)

                    if (ch + 1) % CPS == 0 and ch + 1 < NCHUNK:
                        nc.vector.memset(S_all[:].bitcast(F32), 0.0)
                    if ch + 1 < NCHUNK:
                        nc.vector.tensor_copy(S_b[:], S_all[:])

                # gated product + stats + out_proj for this token tile
                tok0 = nt * NT
                zt = p3f.tile([128, 4, NT], BF16, tag="zt", name="zt")
                nc.sync.dma_start(zt[:], scr['z'][:, tok0:tok0 + NT]
                                  .rearrange("(pt p) n -> p pt n", p=128))
                yz_all = p3f.tile([128, 4, NT], BF16, tag="yzall", name="yzall")
                ssq_full = p3ps.tile([128, NT], F32, tag="mt3", name="mt3ssq")
                ssq_ps = ssq_full[0:1, :]
                for pt in range(4):
                    sz = p3f.tile([128, NT], BF16, tag="sz", name="sz")
                    silu(sz[:], zt[:, pt, :], pool=p3f, tag="szsig")
                    nc.vector.tensor_tensor(yz_all[:, pt, :], y_sb[:, pt, :],
                                            sz[:], Op.mult)
                    sqz = p3f.tile([128, NT], F32R, tag="sqz", name="sqz")
                    nc.scalar.activation(sqz[:], yz_all[:, pt, :], AF.Square)
                    mm(ssq_ps[:], ones128[:], sqz[:],
                       start=(pt == 0), stop=(pt == 3))
                nc.any.tensor_copy(ssq_yz_row[:, tok0:tok0 + NT], ssq_ps[:])

                for mi in range(16):
                    ps = p3ps.tile([128, NT], F32, tag="mt3", name="mt3")
                    for k in range(4):
                        mm(ps[:], w_out_t[:, k, mi * 128:(mi + 1) * 128],
                           yz_all[:, k, :], start=(k == 0), stop=(k == 3))
                    ot = p3f.tile([128, NT], BF16, tag="ot", name="ot")
                    nc.any.tensor_copy(ot[:], ps[:])
                    nc.sync.dma_start(
                        scr['ar1_in8'][nt][mi * 128:(mi + 1) * 128, :], ot[:])

                # inline chunked collectives: tiny stats AR then the big AR
                nc.sync.dma_start(scr['ssq_in8'][nt],
                                  ssq_yz_row[:, tok0:tok0 + NT])
                allreduce(scr['ssq_in8'][nt], scr['ssq_out8'][nt])
                allreduce(scr['ar1_in8'][nt], scr['ar1_out8'][nt])

                if nt == max(0, n8 - 3):
                    # Phase-4 prep for tile 0, overlapped with the P2 tail.
                    # Result (mtn for tile 0) is bounced via DRAM.
                    mt0 = p2pre.tile([128, 16, NT], BF16, tag="mt0",
                                     name="mt0")
                    nc.sync.dma_start(mt0[:], scr['ar1_out8'][0]
                                      .rearrange("(kt p) n -> p kt n", p=128))
                    sqt0 = p2s.tile([1, NT], F32, tag="sqt0", name="sqt0")
                    nc.sync.dma_start(sqt0[:], scr['ssq_out8'][0])
                    sql0 = p2s.tile([1, NT], F32, tag="sql0", name="sql0")
                    nc.scalar.activation(sql0[:], sqt0[:], AF.Ln,
                                         bias=eps1[:], scale=float(1.0 / DIN))
                    s3r0 = p2s.tile([1, NT], F32R, tag="s3r0", name="s3r0")
                    nc.scalar.activation(s3r0[:], sql0[:], AF.Exp, scale=-0.5)
                    bps = p3ps.tile([128, NT], F32, tag="mt3", name="mt3s3b")
                    mm(bps[:], ones1[:], s3r0[:], start=True, stop=True)
                    s3b0 = p2pre.tile([128, NT], BF16, tag="s3b0", name="s3b0")
                    nc.any.tensor_copy(s3b0[:], bps[:])
                    sqf = p3ps.tile([128, NT], F32, tag="mt3", name="mt3ssq0")
                    for k in range(16):
                        ht0 = p2.tile([128, NT], BF16, tag="ht0", name="ht0")
                        nc.sync.dma_start(ht0[:],
                                          io['hsT'][k * 128:(k + 1) * 128,
                                                    0:NT])
                        nc.vector.tensor_tensor(mt0[:, k, :], mt0[:, k, :],
                                                s3b0[:], Op.mult)
                        nc.vector.tensor_tensor(mt0[:, k, :], mt0[:, k, :],
                                                ht0[:], Op.add)
                        nc.sync.dma_start(
                            io['resid2T'][k * 128:(k + 1) * 128, 0:NT],
                            mt0[:, k, :])
                        sq0 = p2s.tile([128, NT], F32R, tag="sq0", name="sq0")
                        nc.scalar.activation(sq0[:], mt0[:, k, :], AF.Square)
                        mm(sqf[0:1, :], ones128[:], sq0[:],
                           start=(k == 0), stop=(k == 15))
                    slr0 = p2s.tile([1, NT], F32, tag="slr0", name="slr0")
                    nc.scalar.activation(slr0[:], sqf[0:1, :], AF.Ln,
                                         bias=eps1[:], scale=float(1.0 / H))
                    sr_0 = p2s.tile([1, NT], F32R, tag="sr_0", name="sr_0")
                    nc.scalar.activation(sr_0[:], slr0[:], AF.Exp, scale=-0.5)
                    sbp0 = p3ps.tile([128, NT], F32, tag="mt3", name="mt3sb0")
                    mm(sbp0[:], ones1[:], sr_0[:], start=True, stop=True)
                    sb0 = p2pre.tile([128, NT], BF16, tag="sb0", name="sb0")
                    nc.any.tensor_copy(sb0[:], sbp0[:])
                    mtn0 = p2pre.tile([128, 16, NT], BF16, tag="mtn0",
                                      name="mtn0")
                    for k in range(16):
                        nc.vector.scalar_tensor_tensor(
                            mtn0[:, k, :], mt0[:, k, :], C['ln2_c'][:, k:k + 1],
                            sb0[:], Op.mult, Op.mult)
                    nc.sync.dma_start(
                        scr['mtn0'].rearrange("(kt p) n -> p kt n", p=128),
                        mtn0[:])

        rows_a_es.close()

        # ================================= Phase 4: resid + ln2 + MLP + RS
        # Software-pipelined: tile j+1's resid/stats/mtn prep is emitted
        # between tile j's gate_up and down matmuls; tile 0's prep ran in
        # Phase 2 (bounced via scr['mtn0']).
        with tc.tile_pool(name="p4w", bufs=1) as p4w, \
             tc.tile_pool(name="p4", bufs=2) as p4, \
             tc.tile_pool(name="p4mt", bufs=1) as p4mt, \
             tc.tile_pool(name="p4row", bufs=1) as p4row, \
             tc.tile_pool(name="p4av", bufs=1) as p4av, \
             tc.tile_pool(name="p4ps_s", bufs=1, space="PSUM") as p4ps_s, \
             tc.tile_pool(name="p4ps_g", bufs=2, space="PSUM") as p4ps_g, \
             tc.tile_pool(name="p4ps_d", bufs=2, space="PSUM") as p4ps_d:
            # tile 0's mtn (precomputed in Phase 2) is fetched before the
            # weights so the first gate matmul isn't stuck behind 12MB of
            # weight DMA on the queue
            mtn_cur = p4.tile([128, 16, NT], BF16, tag="mtn", name="mtn")
            nc.sync.dma_start(mtn_cur[:], scr['mtn0']
                              .rearrange("(kt p) n -> p kt n", p=128))
            wg_t = p4w.tile([128, 16, FFr], BF16, tag="wg_t", name="wg_t")
            nc.sync.dma_start(wg_t[:],
                              io['w_gate'].rearrange("(kt p) m -> p kt m", p=128))
            wu_t = p4w.tile([128, 16, FFr], BF16, tag="wu_t", name="wu_t")
            nc.sync.dma_start(wu_t[:],
                              io['w_up'].rearrange("(kt p) m -> p kt m", p=128))
            wd_t = p4w.tile([128, 8, H], BF16, tag="wd_t", name="wd_t")
            nc.sync.dma_start(wd_t[:],
                              io['w_down'].rearrange("(kt p) m -> p kt m", p=128))

            def p4_prep_a(j):
                # DMA issue + s3 scale chain only: no heavy PE work, so this
                # can sit between gate and down without head-of-line blocking
                # the tensor queue.
                tok0 = j * NT
                mt = p4mt.tile([128, 16, NT], BF16, tag="mt", name="mt")
                nc.sync.dma_start(mt[:], scr['ar1_out8'][j]
                                  .rearrange("(kt p) n -> p kt n", p=128))
                ht_all = p4mt.tile([128, 16, NT], BF16, tag="ht_all",
                                   name="ht_all")
                nc.sync.dma_start(ht_all[:], io['hsT'][:, tok0:tok0 + NT]
                                  .rearrange("(kt p) n -> p kt n", p=128))
                ssq_t = p4row.tile([1, NT], F32, tag="ssq_t", name="ssq_t")
                nc.sync.dma_start(ssq_t[:], scr['ssq_out8'][j])
                ssq_l = p4row.tile([1, NT], F32, tag="ssq_l", name="ssq_l")
                nc.scalar.activation(ssq_l[:], ssq_t[:], AF.Ln,
                                     bias=eps1[:], scale=float(1.0 / DIN))
                s3_row = p4row.tile([1, NT], F32R, tag="s3row", name="s3row")
                nc.scalar.activation(s3_row[:], ssq_l[:], AF.Exp, scale=-0.5)
                s3b_ps = p4ps_s.tile([128, NT], F32, tag="bps", name="s3bps")
                mm(s3b_ps[:], ones1[:], s3_row[:], start=True, stop=True)
                s3b = p4.tile([128, NT], BF16, tag="s3b", name="s3b")
                nc.any.tensor_copy(s3b[:], s3b_ps[:])
                return mt, ht_all, s3b

            def p4_prep_b(j, pa):
                # the PE-heavy stats tail, emitted AFTER this tile's down
                # matmuls so those never stall behind it in the queue
                mt, ht_all, s3b = pa
                tok0 = j * NT
                ssq_ps = p4ps_s.tile([1, NT], F32, tag="ssq", name="ssq")
                for k in range(16):
                    nc.vector.tensor_tensor(mt[:, k, :], mt[:, k, :], s3b[:],
                                            Op.mult)
                    nc.vector.tensor_tensor(mt[:, k, :], mt[:, k, :],
                                            ht_all[:, k, :], Op.add)
                    nc.sync.dma_start(
                        io['resid2T'][k * 128:(k + 1) * 128, tok0:tok0 + NT],
                        mt[:, k, :])
                    sq = p4.tile([128, NT], F32R, tag="sq", name="sq")
                    nc.scalar.activation(sq[:], mt[:, k, :], AF.Square)
                    mm(ssq_ps[:], ones128[:], sq[:],
                       start=(k == 0), stop=(k == 15))
                sr0 = p4row.tile([1, NT], F32, tag="sr0", name="sr0")
                nc.scalar.activation(sr0[:], ssq_ps[:], AF.Ln,
                                     bias=eps1[:], scale=float(1.0 / H))
                s_row = p4row.tile([1, NT], F32R, tag="srow", name="srow")
                nc.scalar.activation(s_row[:], sr0[:], AF.Exp, scale=-0.5)
                sb_ps = p4ps_s.tile([128, NT], F32, tag="bps", name="sbps")
                mm(sb_ps[:], ones1[:], s_row[:], start=True, stop=True)
                sb = p4.tile([128, NT], BF16, tag="sb", name="sb")
                nc.any.tensor_copy(sb[:], sb_ps[:])
                mtn = p4.tile([128, 16, NT], BF16, tag="mtn", name="mtn")
                for k in range(16):
                    nc.vector.scalar_tensor_tensor(
                        mtn[:, k, :], mt[:, k, :], C['ln2_c'][:, k:k + 1],
                        sb[:], Op.mult, Op.mult)
                return mtn

            for nt in range(n8):
                tok0 = nt * NT
                # next tile's prep DMAs + s3 chain are issued FIRST so the
                # data lands while this tile's gate/up matmuls stream
                pa_next = p4_prep_a(nt + 1) if nt + 1 < n8 else None
                # gate_up + silu*up (av kept in SBUF as down-proj k-tiles)
                av = p4av.tile([128, 8, NT], BF16, tag="av", name="av")
                for mi in range(8):
                    gp = p4ps_g.tile([128, NT], F32, tag="gp", name="gp")
                    up = p4ps_g.tile([128, NT], F32, tag="up", name="up")
                    for k in range(16):
                        mm(gp[:], wg_t[:, k, mi * 128:(mi + 1) * 128],
                           mtn_cur[:, k, :], start=(k == 0), stop=(k == 15))
                    for k in range(16):
                        mm(up[:], wu_t[:, k, mi * 128:(mi + 1) * 128],
                           mtn_cur[:, k, :], start=(k == 0), stop=(k == 15))
                    sg = p4.tile([128, NT], BF16, tag="sg", name="sg")
                    silu(sg[:], gp[:], pool=p4, tag="sgsig")
                    nc.vector.tensor_tensor(av[:, mi, :], sg[:], up[:], Op.mult)
                # stats + mtn chain lands between up and down: its DVE/ACT
                # deps were satisfied during the gate/up stream, and the mtn
                # stt chain hides under the down matmuls
                mtn_next = p4_prep_b(nt + 1, pa_next) if pa_next else None
                # down proj -> ReduceScatter chunk (host concats slices).
                # The LAST tile is split into four token-quarters so the
                # end-of-kernel collective tail shrinks to ~one quarter RS.
                if nt < n8 - 1:
                    for mo in range(16):
                        ps = p4ps_d.tile([128, NT], F32, tag="dps", name="dps")
                        for k in range(8):
                            mm(ps[:], wd_t[:, k, mo * 128:(mo + 1) * 128],
                               av[:, k, :], start=(k == 0), stop=(k == 7))
                        ot = p4.tile([128, NT], BF16, tag="ot4", name="ot4")
                        nc.any.tensor_copy(ot[:], ps[:])
                        nc.sync.dma_start(
                            scr['rs2_in8'][nt][mo * 128:(mo + 1) * 128, :],
                            ot[:])
                    if world > 1:
                        nc.gpsimd.collective_compute(
                            "ReduceScatter", Op.add,
                            replica_groups=[list(range(world))],
                            ins=[scr['rs2_in8'][nt]],
                            outs=[scr['rs2_out8'][nt]])
                    else:
                        nc.sync.dma_start(scr['rs2_out8'][nt],
                                          scr['rs2_in8'][nt][0:H // world, :])
                    nc.sync.dma_start(io['out1T'][:, tok0:tok0 + NT],
                                      scr['rs2_out8'][nt])
                else:
                    NH4 = NT // 4
                    for qr in range(4):
                        c0 = qr * NH4
                        for mo in range(16):
                            ps = p4ps_d.tile([128, NH4], F32, tag="dps",
                                             name="dps")
                            for k in range(8):
                                mm(ps[:], wd_t[:, k, mo * 128:(mo + 1) * 128],
                                   av[:, k, c0:c0 + NH4],
                                   start=(k == 0), stop=(k == 7))
                            ot = p4.tile([128, NH4], BF16, tag="ot4",
                                         name="ot4")
                            nc.any.tensor_copy(ot[:], ps[:])
                            nc.sync.dma_start(
                                scr['rs2h_in'][qr][mo * 128:(mo + 1) * 128,
                                                   :], ot[:])
                        if world > 1:
                            nc.gpsimd.collective_compute(
                                "ReduceScatter", Op.add,
                                replica_groups=[list(range(world))],
                                ins=[scr['rs2h_in'][qr]],
                                outs=[scr['rs2h_out'][qr]])
                        else:
                            nc.sync.dma_start(
                                scr['rs2h_out'][qr],
                                scr['rs2h_in'][qr][0:H // world, :])
                        nc.sync.dma_start(
                            io['out1T'][:, tok0 + c0:tok0 + c0 + NH4],
                            scr['rs2h_out'][qr])
                mtn_cur = mtn_next


# ================================================================ entry point
def kernel(**inputs):
    from concourse import bass_utils

    nc = build(world=TP, debug=False)
    in_maps = [shard_core_inputs(inputs, r) for r in range(TP)]
    res = bass_utils.run_bass_kernel_spmd(nc, in_maps, core_ids=list(range(TP)))
    out1T = np.concatenate(
        [np.asarray(res.results[r]['out1T'], dtype=np.float32)
         for r in range(TP)], axis=0)                # [H, T] feature-major
    out1 = np.ascontiguousarray(out1T.T).reshape(B, L, H)
    resid2 = np.ascontiguousarray(
        np.asarray(res.results[0]['resid2T'], dtype=np.float32).T
    ).reshape(B, L, H)
    return out1, resid2


if __name__ == '__main__':
    nc = build(world=1)
    print("built ok")



# revision 20
# speedup vs baseline: 1.0307x; 1.0288x over previous
"""Trainium2 Bass kernel for nn_BambaMixerDecoderLayer_84696755077458.

Tensor-parallel over 8 NeuronCores (vLLM-style), v3 (merged pipeline):
  - in_proj / gate_up column-sharded, out_proj / down row-sharded
  - heads + conv channels sharded with d_inner; B/C conv streams replicated
  - SSM scan via chunked SSD (Mamba2): group-of-4 segsum built in PSUM fp32
    (batched col/tri matmuls at N=512), per-head fp32r broadcasts; bf16
    moving operands on all y matmuls (fp32 state master + bf16 shadow).
  - in_proj (tile nt) emission is INTERLEAVED with SSD chunks + gated-norm +
    out_proj of tile nt-1, so the PE stream never gaps: the tensor clock
    stays ramped and the SSD dependency chain hides under in_proj matmuls.
    Conv/x/z streams stay in SBUF (no DRAM bounce).
  - AllReduce chunked per 512-token tile and issued inline starting ~100us
    into the kernel; rmsnorm sum-of-squares rides in 2 batched fp32 ARs.
  - MLP phase: prep DMAs for tile j+1 issued before tile j's gate/up, stats
    matmuls emitted between up and down (no tensor-queue HOL), mtn chain
    hidden under down; last tile split in token halves to shrink the
    ReduceScatter tail.
Everything on-device is feature-major ([feature, token]); host does layout
transforms (transpose / shard / concat) only.

Self-contained: hardcodes all shapes; needs only /opt/trn_rl_repo on sys.path.
"""
import sys
from contextlib import ExitStack

if '/opt/trn_rl_repo' not in sys.path:
    sys.path.insert(0, '/opt/trn_rl_repo')

import numpy as np

# ---------------------------------------------------------------- constants
H = 2048          # hidden
DIN = 4096        # mamba intermediate
DS = 128          # ssm state
DCONV = 4
NH = 64
HD = 64
FF = 8192
EPS = 1e-5
B, L = 2, 2048
T = B * L                         # 4096 tokens
CONV_DIM = DIN + 2 * DS           # 4352
D_IN_PROJ = 2 * DIN + 2 * DS + NH  # 8512

TP = 8
NHr = NH // TP                    # 8 heads / core
DINr = DIN // TP                  # 512
FFr = FF // TP                    # 1024
CONVr = DINr + 2 * DS             # 768 conv channels / core
MPROJ = DINr + CONVr + NHr        # 1288 in_proj cols / core

Q = 128                           # SSD chunk
NT = 512                          # token tile (also the collective chunk)
NEG = -3.0e38
SIM_SILU = False   # True: emit sigmoid+mul instead of Silu (CoreSim support)


def _f32(x):
    return np.ascontiguousarray(np.asarray(x, dtype=np.float32))


def _bf16(x):
    import ml_dtypes
    return np.ascontiguousarray(
        np.asarray(x, dtype=np.float32).astype(ml_dtypes.bfloat16))


# ================================================================ host prep
def host_constants():
    import ml_dtypes
    identb = np.eye(128, dtype=ml_dtypes.bfloat16)
    i8 = np.eye(8, dtype=np.float32)
    sel8 = np.zeros((8, 8 * 128), np.float32)
    for h in range(8):
        sel8[h, h * 128:(h + 1) * 128] = 1.0
    # negsel4[4g+i, g*512 + i*128 + q'] = -1  (col term for group-of-4 segsum)
    negsel4 = np.zeros((8, 1024), np.float32)
    for h in range(8):
        negsel4[h, h * 128:(h + 1) * 128] = -1.0
    ones8 = np.ones((8, 128), np.float32)
    ones1 = np.ones((1, 128), np.float32)
    ones128 = np.ones((128, 1), np.float32)
    tri = np.where(np.arange(Q)[:, None] > np.arange(Q)[None, :], NEG, 0.0)
    trimask4 = np.concatenate([tri] * 4, axis=1).astype(ml_dtypes.bfloat16)
    return dict(c_identb=identb, c_i8=i8, c_sel8=sel8,
                c_sel8b=sel8.astype(ml_dtypes.bfloat16), c_negsel4=negsel4,
                c_ones8=ones8, c_ones1=ones1, c_ones128=ones128,
                c_ones128b=ones128.astype(ml_dtypes.bfloat16),
                c_trimask4=trimask4)


def shard_core_inputs(inputs, r):
    """Build the per-core input map (all feature-major)."""
    w_in = _f32(inputs['w_in'])
    zs = slice(DINr * r, DINr * (r + 1))
    xs = slice(DIN + DINr * r, DIN + DINr * (r + 1))
    bs = slice(2 * DIN, 2 * DIN + DS)
    cs = slice(2 * DIN + DS, 2 * DIN + 2 * DS)
    dts = slice(2 * DIN + 2 * DS + NHr * r, 2 * DIN + 2 * DS + NHr * (r + 1))
    w_in_r = np.concatenate(
        [w_in[:, zs], w_in[:, xs], w_in[:, bs], w_in[:, cs], w_in[:, dts]], axis=1)

    conv_w = _f32(inputs['conv_w'])
    conv_w_r = np.concatenate([conv_w[DINr * r:DINr * (r + 1)], conv_w[DIN:]], axis=0)
    conv_b = _f32(inputs['conv_b'])
    conv_b_r = np.concatenate([conv_b[DINr * r:DINr * (r + 1)], conv_b[DIN:]], axis=0)

    hs = _f32(inputs['hidden_states'])
    hs = hs.reshape(-1, H)

    A_r = _f32(inputs['A_log'])[NHr * r:NHr * (r + 1)]
    dtb_r = _f32(inputs['dt_bias'])[NHr * r:NHr * (r + 1)]
    D_r = _f32(inputs['D_ssm'])[NHr * r:NHr * (r + 1)]

    m = dict(host_constants())
    m['hsT'] = _bf16(hs.T)                                      # [2048, T]
    m['w_in'] = _bf16(w_in_r)                                   # [2048, 1288]
    # per-k-tile column form of per-feature vectors: [128, n_tiles]
    m['ln1_c'] = np.ascontiguousarray(_f32(inputs['ln1_w']).reshape(16, 128).T)
    m['ln2_c'] = np.ascontiguousarray(_f32(inputs['ln2_w']).reshape(16, 128).T)
    m['normw_c'] = np.ascontiguousarray(
        _f32(inputs['norm_w'])[DINr * r:DINr * (r + 1)].reshape(4, 128).T)
    m['dssm_c'] = np.ascontiguousarray(
        np.repeat(D_r, HD).reshape(4, 128).T)                   # [128, 4]
    # conv weights: [128, 6*4] with [p, pt*4+d]
    m['conv_w'] = np.ascontiguousarray(
        conv_w_r.reshape(6, 128, DCONV).transpose(1, 0, 2).reshape(128, 6 * DCONV))
    m['conv_b'] = np.ascontiguousarray(conv_b_r.reshape(6, 128).T)  # [128, 6]
    m['a_col'] = np.ascontiguousarray((-np.exp(A_r))[:, None])   # [8,1]
    m['dtb_col'] = np.ascontiguousarray(dtb_r[:, None])          # [8,1]
    m['w_out'] = _bf16(_f32(inputs['w_out'])[DINr * r:DINr * (r + 1)])
    wgu = _f32(inputs['w_gate_up'])
    m['w_gate'] = _bf16(wgu[:, FFr * r:FFr * (r + 1)])
    m['w_up'] = _bf16(wgu[:, FF + FFr * r:FF + FFr * (r + 1)])
    m['w_down'] = _bf16(_f32(inputs['w_down'])[FFr * r:FFr * (r + 1)])
    return m


# ================================================================ the kernel
def build(world=TP, debug=False, T_=T):
    import concourse.mybir as mybir
    import concourse.tile as tile
    from concourse import bacc
    from concourse.alu_op_type import AluOpType as Op

    AF = mybir.ActivationFunctionType
    F32 = mybir.dt.float32
    BF16 = mybir.dt.bfloat16

    nc = bacc.Bacc("TRN2", target_bir_lowering=False, debug=False,
                   num_devices=world)

    F32R = mybir.dt.float32r
    n8 = T_ // NT

    def din(name, shape, dt):
        return nc.dram_tensor(name, list(shape), dt, kind="ExternalInput").ap()

    BIN = {'hsT', 'w_in', 'w_out', 'w_gate', 'w_up', 'w_down', 'c_identb',
           'c_trimask4', 'c_sel8b', 'c_ones128b'}
    RIN = {'c_i8', 'c_sel8', 'c_negsel4', 'c_ones8', 'c_ones1', 'c_ones128'}
    io = {}
    for name, shape in [
            ('hsT', (H, T_)), ('w_in', (H, MPROJ)),
            ('ln1_c', (128, 16)), ('ln2_c', (128, 16)),
            ('normw_c', (128, 4)), ('dssm_c', (128, 4)),
            ('conv_w', (128, 24)), ('conv_b', (128, 6)),
            ('a_col', (8, 1)), ('dtb_col', (8, 1)),
            ('w_out', (DINr, H)), ('w_gate', (H, FFr)), ('w_up', (H, FFr)),
            ('w_down', (FFr, H)),
            ('c_identb', (128, 128)), ('c_i8', (8, 8)), ('c_sel8', (8, 1024)),
            ('c_sel8b', (8, 1024)), ('c_negsel4', (8, 1024)),
            ('c_ones8', (8, 128)),
            ('c_ones1', (1, 128)), ('c_ones128', (128, 1)),
            ('c_ones128b', (128, 1)),
            ('c_trimask4', (128, 512))]:
        dt = BF16 if name in BIN else (F32R if name in RIN else F32)
        io[name] = din(name, shape, dt)

    io['out1T'] = nc.dram_tensor("out1T", [H // world, T_], BF16,
                                 kind="ExternalOutput").ap()
    io['resid2T'] = nc.dram_tensor("resid2T", [H, T_], BF16,
                                   kind="ExternalOutput").ap()

    scr = {}
    scr['ar1_in8'] = [
        nc.dram_tensor(f"ar1_in{q}", [H, NT], BF16, kind="Internal").ap()
        for q in range(n8)]
    scr['ar1_out8'] = [
        nc.dram_tensor(f"ar1_out{q}", [H, NT], BF16, kind="Internal",
                       addr_space="Shared").ap() for q in range(n8)]
    scr['ssq_in'] = nc.dram_tensor("ssq_in", [1, T_], F32,
                                   kind="Internal").ap()
    scr['ssq_out'] = nc.dram_tensor("ssq_out", [1, T_], F32, kind="Internal",
                                    addr_space="Shared").ap()
    scr['rs2_in8'] = [
        nc.dram_tensor(f"rs2_in{q}", [H, NT], BF16, kind="Internal").ap()
        for q in range(n8)]
    scr['rs2_out8'] = [
        nc.dram_tensor(f"rs2_out{q}", [H // world, NT], BF16,
                       kind="Internal").ap() for q in range(n8)]
    scr['rs2h_in'] = [
        nc.dram_tensor(f"rs2h_in{q}", [H, NT // 2], BF16,
                       kind="Internal").ap() for q in range(2)]
    scr['rs2h_out'] = [
        nc.dram_tensor(f"rs2h_out{q}", [H // world, NT // 2], BF16,
                       kind="Internal").ap() for q in range(2)]

    with tile.TileContext(nc) as tc:
        _body(tc, io, scr, world, debug, mybir, tile, Op, AF, F32, T_)

    nc.compile()
    return nc


def _body(tc, io, scr, world, debug, mybir, tile, Op, AF, F32, T_):
    nc = tc.nc
    F32R = mybir.dt.float32r
    BF16 = mybir.dt.bfloat16
    n8 = T_ // NT
    NCHUNK = T_ // Q
    CPS = (T_ // B) // Q          # chunks per sequence

    def mm(out, lhsT, rhs, start, stop, skip=False):
        if lhsT.dtype == F32:
            lhsT = lhsT.bitcast(F32R)
        if rhs.dtype == F32:
            rhs = rhs.bitcast(F32R)
        nc.tensor.matmul(out, lhsT, rhs, start=start, stop=stop,
                         skip_group_check=skip)

    def silu(out_ap, in_ap, bias=0.0, pool=None, tag="silu_tmp"):
        if SIM_SILU:
            tmp = pool.tile(list(out_ap.shape), F32, tag=tag, name=tag)
            nc.scalar.activation(tmp[:], in_ap, AF.Sigmoid, bias=bias, scale=1.0)
            if isinstance(bias, float) and bias == 0.0:
                nc.vector.tensor_tensor(out_ap, in_ap, tmp[:], Op.mult)
            else:
                raise NotImplementedError("SIM_SILU with bias AP")
        else:
            nc.scalar.activation(out_ap, in_ap, AF.Silu, bias=bias, scale=1.0)

    def allreduce(in_ap, out_ap):
        if world > 1:
            nc.gpsimd.collective_compute(
                "AllReduce", Op.add, replica_groups=[list(range(world))],
                ins=[in_ap], outs=[out_ap])
        else:
            nc.sync.dma_start(out_ap, in_ap)

    with ExitStack() as ES:
        cpool = ES.enter_context(tc.tile_pool(name="consts", bufs=1))

        # -------------------------------------------------------- constants
        C = {}
        RT = {'c_i8', 'c_sel8', 'c_negsel4', 'c_ones8', 'c_ones1',
              'c_ones128'}
        BT = {'c_identb', 'c_trimask4', 'c_sel8b', 'c_ones128b'}
        for nm, shape in [('c_identb', (128, 128)), ('c_i8', (8, 8)),
                          ('c_sel8', (8, 1024)), ('c_sel8b', (8, 1024)),
                          ('c_negsel4', (8, 1024)),
                          ('c_ones8', (8, 128)), ('c_ones1', (1, 128)),
                          ('c_ones128', (128, 1)), ('c_ones128b', (128, 1)),
                          ('c_trimask4', (128, 512)),
                          ('ln1_c', (128, 16)), ('ln2_c', (128, 16)),
                          ('normw_c', (128, 4)), ('dssm_c', (128, 4)),
                          ('conv_w', (128, 24)), ('conv_b', (128, 6)),
                          ('a_col', (8, 1)), ('dtb_col', (8, 1))]:
            dt = BF16 if nm in BT else (F32R if nm in RT else F32)
            t = cpool.tile(list(shape), dt, tag=nm)
            nc.sync.dma_start(t[:], io[nm])
            C[nm] = t
        identb, i8 = C['c_identb'], C['c_i8']
        sel8, negsel4 = C['c_sel8'], C['c_negsel4']
        sel8b = C['c_sel8b']
        ones8, ones1, ones128 = C['c_ones8'], C['c_ones1'], C['c_ones128']
        ones128b = C['c_ones128b']
        trimask4 = C['c_trimask4']

        eps1 = cpool.tile([1, 1], F32, tag="eps1", name="eps1")
        nc.vector.memset(eps1[:], float(EPS))

        # ============================== merged in_proj + conv + SSD + out
        with tc.tile_pool(name="p1w", bufs=1) as p1w, \
             tc.tile_pool(name="p1h", bufs=2) as p1h, \
             tc.tile_pool(name="sxz", bufs=2) as sxz, \
             tc.tile_pool(name="cvt", bufs=2) as cvt, \
             tc.tile_pool(name="cvx", bufs=1) as cvx, \
             tc.tile_pool(name="p2s", bufs=2) as p2s, \
             tc.tile_pool(name="p1s", bufs=2) as p1s, \
             tc.tile_pool(name="pdt", bufs=1) as pdt, \
             tc.tile_pool(name="prow", bufs=2) as prow, \
             tc.tile_pool(name="spool", bufs=1) as spool, \
             tc.tile_pool(name="p3w", bufs=1) as p3w, \
             tc.tile_pool(name="psM", bufs=2, space="PSUM") as psM, \
             tc.tile_pool(name="psS", bufs=1, space="PSUM") as psS, \
             tc.tile_pool(name="psC", bufs=1, space="PSUM") as psC, \
             tc.tile_pool(name="psG", bufs=1, space="PSUM") as psG:

            # hst tile 0 prefetched before weights so stats matmuls can warm
            # up the PE while w_in streams in
            hst0 = p1h.tile([128, 16, NT], BF16, tag="hst", name="hst")
            nc.sync.dma_start(hst0[:, 0:8, :], io['hsT'][0:8 * 128, 0:NT]
                              .rearrange("(kt p) n -> p kt n", p=128))
            nc.sync.dma_start(hst0[:, 8:16, :], io['hsT'][8 * 128:H, 0:NT]
                              .rearrange("(kt p) n -> p kt n", p=128))
            w1 = p1w.tile([128, 16, MPROJ], BF16, tag="w1", name="w1")
            nc.sync.dma_start(
                w1[:], io['w_in'].rearrange("(kt p) m -> p kt m", p=128))
            for k in range(16):
                nc.vector.tensor_scalar_mul(w1[:, k, :], w1[:, k, :],
                                            C['ln1_c'][:, k:k + 1])
            w_out_t = p3w.tile([128, 4, H], BF16, tag="w_out_t",
                               name="w_out_t")
            nc.sync.dma_start(w_out_t[:],
                              io['w_out'].rearrange("(kt p) m -> p kt m",
                                                    p=128))
            for k in range(4):
                nc.vector.tensor_scalar_mul(w_out_t[:, k, :], w_out_t[:, k, :],
                                            C['normw_c'][:, k:k + 1])

            S_all = spool.tile([128, NHr * HD], F32R, tag="S_all",
                               name="S_all")
            nc.vector.memset(S_all[:].bitcast(F32), 0.0)
            # bf16 shadow of the state for the y_inter matmul (PE cannot mix
            # f32r weights with bf16 moving); fp32 master stays exact
            S_b = spool.tile([128, NHr * HD], BF16, tag="S_b", name="S_b")
            nc.vector.memset(S_b[:], 0.0)

            # ---------------- per-tile emit pieces -------------------------
            TS = {}           # state shared between pieces of one tile

            def emit_stats(nt):
                """ln1 sum-of-squares + rms scale for tile nt."""
                tok0 = nt * NT
                if nt == 0:
                    hst = hst0
                else:
                    hst = p1h.tile([128, 16, NT], BF16, tag="hst", name="hst")
                    nc.sync.dma_start(hst[:], io['hsT'][:, tok0:tok0 + NT]
                                      .rearrange("(kt p) n -> p kt n", p=128))
                ssq_ps = psS.tile([1, NT], F32, tag="ssq", name="ssq")
                for k in range(16):
                    sq = p1s.tile([128, NT], BF16, tag="sq", name="sq")
                    nc.scalar.activation(sq[:], hst[:, k, :], AF.Square)
                    mm(ssq_ps[:], ones128b[:], sq[:],
                       start=(k == 0), stop=(k == 15))
                sr0 = pdt.tile([1, NT], F32, tag="sr0", name="sr0")
                nc.scalar.activation(sr0[:], ssq_ps[:], AF.Ln,
                                     bias=eps1[:], scale=float(1.0 / H))
                s_row = p1s.tile([1, NT], F32R, tag="s_row", name="s_row")
                nc.scalar.activation(s_row[:], sr0[:], AF.Exp, scale=-0.5)
                sb_ps = psM.tile([128, NT], F32, tag="mt", name="sbps")
                mm(sb_ps[:], ones1[:], s_row[:], start=True, stop=True)
                sb = p1s.tile([128, NT], F32, tag="sb", name="sb")
                nc.any.tensor_copy(sb[:], sb_ps[:])
                TS['hst'], TS['sb'] = hst, sb
                TS['z_sb'] = sxz.tile([128, 4, NT], BF16, tag="z_sb",
                                      name="z_sb")
                TS['xbc'] = cvx.tile([128, 6, NT + 3], BF16, tag="xbc",
                                     name="xbc")
                TS['xconv'] = sxz.tile([128, 6, NT], BF16, tag="xconv",
                                       name="xconv")
                TS['tail'] = cvt.tile([128, 6, 3], BF16, tag="cvtail",
                                      name="cvtail")

            def emit_mz(nt, mi):
                """z m-tile mi of tile nt."""
                hst, sb = TS['hst'], TS['sb']
                ps = psM.tile([128, NT], F32, tag="mt", name="mt")
                for k in range(16):
                    mm(ps[:], w1[:, k, mi * 128:(mi + 1) * 128],
                       hst[:, k, :], start=(k == 0), stop=(k == 15))
                nc.vector.tensor_tensor(TS['z_sb'][:, mi, :], ps[:], sb[:],
                                        Op.mult)

            def emit_mxbc(nt, pt, tail_prev):
                """xBC m-tile pt of tile nt + causal conv + silu."""
                tok0 = nt * NT
                seq_start = (tok0 % (T_ // B)) == 0
                hst, sb = TS['hst'], TS['sb']
                xbc, xconv = TS['xbc'], TS['xconv']
                ps = psM.tile([128, NT], F32, tag="mt", name="mt")
                for k in range(16):
                    mm(ps[:], w1[:, k, DINr + pt * 128:DINr + (pt + 1) * 128],
                       hst[:, k, :], start=(k == 0), stop=(k == 15))
                nc.vector.tensor_tensor(xbc[:, pt, 3:3 + NT], ps[:], sb[:],
                                        Op.mult)
                if seq_start:
                    nc.vector.memset(xbc[:, pt, 0:3], 0.0)
                else:
                    nc.vector.tensor_copy(xbc[:, pt, 0:3],
                                          tail_prev[:, pt, :])
                acc = p1s.tile([128, NT], BF16, tag="cacc", name="cacc")
                nc.vector.tensor_scalar_mul(
                    acc[:], xbc[:, pt, 0:NT],
                    C['conv_w'][:, pt * 4:pt * 4 + 1])
                for dd in range(1, 4):
                    nc.vector.scalar_tensor_tensor(
                        acc[:], xbc[:, pt, dd:dd + NT],
                        C['conv_w'][:, pt * 4 + dd:pt * 4 + dd + 1],
                        acc[:], Op.mult, Op.add)
                if SIM_SILU:
                    nc.vector.tensor_scalar_add(acc[:], acc[:],
                                                C['conv_b'][:, pt:pt + 1])
                    silu(xconv[:, pt, :], acc[:], pool=p1s, tag="cvsig")
                else:
                    nc.scalar.activation(xconv[:, pt, :], acc[:], AF.Silu,
                                         bias=C['conv_b'][:, pt:pt + 1],
                                         scale=1.0)
                nc.vector.tensor_copy(TS['tail'][:, pt, :],
                                      xbc[:, pt, NT:NT + 3])

            def emit_dt(nt):
                """dt m-tile + softplus + per-chunk log-decay cumsum rows."""
                hst, sb = TS['hst'], TS['sb']
                dtp = psS.tile([8, NT], F32, tag="mtdt", name="mtdt")
                for k in range(16):
                    mm(dtp[:], w1[:, k, DINr + CONVr:MPROJ], hst[:, k, :],
                       start=(k == 0), stop=(k == 15))
                dt_raw = pdt.tile([8, NT], F32, tag="dtraw", name="dtraw")
                nc.vector.tensor_tensor(dt_raw[:], dtp[:], sb[:8, :], Op.mult)
                e8 = pdt.tile([8, NT], F32, tag="e8", name="e8")
                nc.scalar.activation(e8[:], dt_raw[:], AF.Exp,
                                     bias=C['dtb_col'][:], scale=1.0)
                nc.vector.tensor_scalar_add(e8[:], e8[:], 1.0)
                dtr = prow.tile([8, NT], F32R, tag="dtr", name="dtr")
                nc.scalar.activation(dtr[:], e8[:], AF.Ln)
                logda = dt_raw
                nc.vector.tensor_scalar_mul(logda[:], dtr[:], C['a_col'][:])
                lar = prow.tile([8, NT], F32R, tag="lar", name="lar")
                for c in range(NT // Q):
                    nc.vector.tensor_tensor_scan(
                        lar[:, c * Q:(c + 1) * Q],
                        ones8[:, :Q].bitcast(F32), logda[:, c * Q:(c + 1) * Q],
                        0.0, Op.mult, Op.add)
                TS['dtr'], TS['lar'] = dtr, lar

            # ---- SSD chunk pieces (operate on tile data in CS dict) ------
            def chunk_front(CS, ch):
                """misc + transposes + segsum matmuls for chunk ch."""
                cc_ = ((ch * Q) % NT) // Q
                xconv = CS['xconv']
                xf = xconv[:, 0:4, cc_ * Q:(cc_ + 1) * Q]
                bf = xconv[:, 4, cc_ * Q:(cc_ + 1) * Q]
                cf = xconv[:, 5, cc_ * Q:(cc_ + 1) * Q]
                lrow = CS['lar'][:, cc_ * Q:(cc_ + 1) * Q]
                dtrow = CS['dtr'][:, cc_ * Q:(cc_ + 1) * Q]

                explb = p2s.tile([8, Q], BF16, tag="explb", name="explb")
                nc.scalar.activation(explb[:], lrow, AF.Exp)
                expl_l = p2s.tile([8, 1], F32, tag="expl_l", name="expl_l")
                nc.scalar.activation(expl_l[:], lrow[:, Q - 1:Q], AF.Exp)
                ddr0 = p2s.tile([8, Q], F32, tag="ddr0", name="ddr0")
                nc.vector.tensor_scalar(ddr0[:], lrow, -1.0,
                                        lrow[:, Q - 1:Q].bitcast(F32),
                                        Op.mult, Op.add)
                dd_rows = p2s.tile([8, Q], F32R, tag="ddrows", name="ddrows")
                nc.scalar.activation(dd_rows[:], ddr0[:], AF.Exp)
                nc.vector.tensor_tensor(dd_rows[:], dd_rows[:], dtrow,
                                        Op.mult)
                dg = p2s.tile([8, 8], F32R, tag="dg", name="dg")
                nc.vector.tensor_scalar_mul(dg[:], i8[:], expl_l[:])

                misc = psC.tile([128, 160], F32, tag="misc", name="misc")
                mm(misc[:, 0:128], bf, cf, start=True, stop=True)
                mm(misc[:, 128:136], dd_rows[:], i8[:], start=True, stop=True)
                mm(misc[:, 136:144], ones8[:], dg[:], start=True, stop=True)
                mm(misc[:, 144:152], dtrow, i8[:], start=True, stop=True)
                g_sb = p2s.tile([128, 128], BF16, tag="g_sb", name="g_sb")
                nc.any.tensor_copy(g_sb[:], misc[:, 0:128])
                dsc = p2s.tile([128, 16], F32, tag="dsc", name="dsc")
                nc.any.tensor_copy(dsc[:], misc[:, 128:144])
                dtc_b = p2s.tile([128, 8], BF16, tag="dtc_b", name="dtc_b")
                nc.any.tensor_copy(dtc_b[:], misc[:, 144:152])

                tps = psC.tile([128, 5, 128], BF16, tag="xtm", name="xtm")
                nc.tensor.transpose(tps[:, 4, :], bf, identb[:])
                btm = p2s.tile([128, Q], BF16, tag="btm", name="btm")
                nc.any.tensor_copy(btm[:], tps[:, 4, :])
                for pt in range(4):
                    nc.tensor.transpose(tps[:, pt, :], xf[:, pt, :],
                                        identb[:])
                xtm = p2s.tile([128, NHr, HD], BF16, tag="xtm_sb",
                               name="xtm_sb")
                nc.any.tensor_copy(
                    xtm[:], tps[:, 0:4, :].rearrange(
                        "p f (h d) -> p (f h) d", d=HD))
                xw = p2s.tile([128, NHr, HD], BF16, tag="xw", name="xw")
                nc.vector.tensor_tensor(
                    xw[:], xtm[:],
                    dsc[:, 0:8][:, :, None].broadcast_to([128, NHr, HD]),
                    Op.mult)

                segs = []
                for g in range(2):
                    seg = psG.tile([128, 2, 4, 128], F32, tag="seg",
                                   name="seg")
                    segs.append(seg)
                    # PSUM zero-region semantics: start=True re-marks the
                    # whole 2KB bank pending-zero, so exactly ONE start per
                    # bank; later first-touch writes overwrite via pending
                    # bits and full-width writes accumulate.
                    for i in range(4):
                        h = 4 * g + i
                        mm(seg[:, 0, i, :],
                           sel8[:, h * 128:(h + 1) * 128], lrow,
                           start=(i == 0), stop=False, skip=True)
                    mm(seg[:, 0], lrow,
                       negsel4[:, g * 512:(g + 1) * 512],
                       start=False, stop=False, skip=True)
                    mm(seg[:, 0], identb[:], trimask4[:],
                       start=False, stop=True, skip=True)
                    for i in range(4):
                        h = 4 * g + i
                        mm(seg[:, 1, i, :],
                           sel8b[:, h * 128:(h + 1) * 128], explb[:],
                           start=(i == 0), stop=(i == 3), skip=True)
                CS.update(segs=segs, xf=xf, cf=cf, xtm=xtm, xw=xw, btm=btm,
                          g_sb=g_sb, dsc=dsc, dtc_b=dtc_b)

            def chunk_back(CS, ch):
                """w0/wt/ce + y matmuls + readout + state update."""
                cc_ = ((ch * Q) % NT) // Q
                segs, xf, cf = CS['segs'], CS['xf'], CS['cf']
                xtm, xw, btm = CS['xtm'], CS['xw'], CS['btm']
                g_sb, dsc, dtc_b = CS['g_sb'], CS['dsc'], CS['dtc_b']
                y_sb = CS['y_sb']
                for g in range(2):
                    seg = segs[g]
                    w0 = p2s.tile([128, 4, 128], BF16, tag="w0", name="w0")
                    nc.scalar.activation(w0[:], seg[:, 0], AF.Exp)
                    wt = p2s.tile([128, 4, 128], BF16, tag="wt", name="wt")
                    nc.vector.tensor_tensor(
                        wt[:], w0[:],
                        dtc_b[:, 4 * g:4 * g + 4, None]
                        .broadcast_to([128, 4, 128]), Op.mult)
                    nc.vector.tensor_tensor(
                        wt[:], wt[:],
                        g_sb[:, None, :].broadcast_to([128, 4, 128]),
                        Op.mult)
                    ce = p2s.tile([128, 4, 128], BF16, tag="ce", name="ce")
                    nc.vector.tensor_tensor(
                        ce[:], seg[:, 1],
                        cf[:, None, :].broadcast_to([128, 4, 128]),
                        Op.mult)
                    # y accumulates into the (already consumed) dquad bank:
                    # head 4g+2j -> partitions 0:64, col block j;
                    # head 4g+2j+1 -> partitions 64:128, col block j
                    for i in range(4):
                        h = 4 * g + i
                        j, half = i // 2, i % 2
                        ysl = seg[64 * half:64 * (half + 1), 0, j, :]
                        mm(ysl, xtm[:, h, :], wt[:, i, :],
                           start=True, stop=False, skip=True)
                        mm(ysl, S_b[:, h * HD:(h + 1) * HD],
                           ce[:, i, :], start=False, stop=True, skip=True)
                    for j in range(2):
                        pt = 2 * g + j
                        nc.vector.scalar_tensor_tensor(
                            y_sb[:, pt, cc_ * Q:(cc_ + 1) * Q],
                            xf[:, pt, :], C['dssm_c'][:, pt:pt + 1],
                            seg[:, 0, j, :], Op.mult, Op.add)

                tp_ps = psM.tile([128, 512], F32, tag="mt", name="tp")
                mm(tp_ps[:], btm[:], xw[:], start=True, stop=True)
                S3 = S_all[:].rearrange("p (h d) -> p h d", d=HD)
                nc.vector.tensor_tensor(
                    S3, S3,
                    dsc[:, 8:16][:, :, None].broadcast_to([128, NHr, HD]),
                    Op.mult)
                nc.vector.tensor_tensor(
                    S3, S3, tp_ps[:].rearrange("p (h d) -> p h d", d=HD),
                    Op.add)
                if (ch + 1) % CPS == 0 and ch + 1 < NCHUNK:
                    nc.vector.memset(S_all[:].bitcast(F32), 0.0)
                if ch + 1 < NCHUNK:
                    nc.vector.tensor_copy(S_b[:], S_all[:])

            def emit_p3(CS, pnt):
                """gated norm + stats + out_proj + inline AR for tile pnt."""
                tok0 = pnt * NT
                y_sb, z_sb = CS['y_sb'], CS['z_sb']
                ssq_ps = psS.tile([1, NT], F32, tag="ssq", name="ssq3")
                for pt in range(4):
                    silu(z_sb[:, pt, :], z_sb[:, pt, :], pool=p1s, tag="szsig")
                    nc.vector.tensor_tensor(y_sb[:, pt, :], y_sb[:, pt, :],
                                            z_sb[:, pt, :], Op.mult)
                    sqz = p1s.tile([128, NT], BF16, tag="sqz", name="sqz")
                    nc.scalar.activation(sqz[:], y_sb[:, pt, :], AF.Square)
                    mm(ssq_ps[:], ones128b[:], sqz[:],
                       start=(pt == 0), stop=(pt == 3))
                sst = p1s.tile([1, NT], F32, tag="sst", name="sst")
                nc.any.tensor_copy(sst[:], ssq_ps[:])
                nc.sync.dma_start(scr['ssq_in'][:, tok0:tok0 + NT], sst[:])

                for mi in range(16):
                    ps = psM.tile([128, NT], F32, tag="mt", name="mt")
                    for k in range(4):
                        mm(ps[:], w_out_t[:, k, mi * 128:(mi + 1) * 128],
                           y_sb[:, k, :], start=(k == 0), stop=(k == 3))
                    ot = p1s.tile([128, NT], BF16, tag="ot", name="ot")
                    nc.any.tensor_copy(ot[:], ps[:])
                    nc.sync.dma_start(
                        scr['ar1_in8'][pnt][mi * 128:(mi + 1) * 128, :],
                        ot[:])
                allreduce(scr['ar1_in8'][pnt], scr['ar1_out8'][pnt])
                # batched rms-stats ARs: [tiles 0..3] and [tiles 4..7]
                half_t = min(4 * NT, T_)
                if pnt == min(3, n8 - 1):
                    allreduce(scr['ssq_in'][:, 0:half_t],
                              scr['ssq_out'][:, 0:half_t])
                if pnt == n8 - 1 and T_ > half_t:
                    allreduce(scr['ssq_in'][:, half_t:T_],
                              scr['ssq_out'][:, half_t:T_])

            # ------------------------- merged schedule --------------------
            prev = None       # CS dict of tile nt-1 (chunks pending)
            tail_prev = None
            for nt in range(n8 + 1):
                if nt < n8:
                    emit_stats(nt)
                    mp = ([lambda mi=mi: emit_mz(nt, mi) for mi in range(4)]
                          + [lambda pt=pt, tp_=tail_prev:
                             emit_mxbc(nt, pt, tp_) for pt in range(6)]
                          + [lambda: emit_dt(nt)])
                else:
                    mp = []
                if prev is not None:
                    CS = prev
                    pnt = nt - 1
                    cps = []
                    for c in range(4):
                        ch = pnt * 4 + c
                        cps.append(lambda ch=ch: chunk_front(CS, ch))
                        cps.append(lambda ch=ch: chunk_back(CS, ch))
                    cps.append(lambda: emit_p3(CS, pnt))
                else:
                    cps = []
                # interleave: chunk piece, m piece, chunk piece, m piece ...
                ia, ib = 0, 0
                while ia < len(cps) or ib < len(mp):
                    if ia < len(cps):
                        cps[ia]()
                        ia += 1
                    if ib < len(mp):
                        mp[ib]()
                        ib += 1
                    if (ib < len(mp)
                            and len(mp) - ib > 2 * (len(cps) - ia)):
                        mp[ib]()
                        ib += 1
                if nt < n8:
                    prev = dict(xconv=TS['xconv'], z_sb=TS['z_sb'],
                                dtr=TS['dtr'], lar=TS['lar'],
                                y_sb=sxz.tile([128, 4, NT], BF16, tag="y_sb",
                                              name="y_sb"))
                    tail_prev = TS['tail']

        # ================================= Phase 4: resid + ln2 + MLP + RS
        with tc.tile_pool(name="p4w", bufs=1) as p4w, \
             tc.tile_pool(name="p4", bufs=2) as p4, \
             tc.tile_pool(name="p4mt", bufs=1) as p4mt, \
             tc.tile_pool(name="p4row", bufs=1) as p4row, \
             tc.tile_pool(name="p4av", bufs=1) as p4av, \
             tc.tile_pool(name="p4ps_s", bufs=1, space="PSUM") as p4ps_s, \
             tc.tile_pool(name="p4ps_g", bufs=2, space="PSUM") as p4ps_g, \
             tc.tile_pool(name="p4ps_d", bufs=2, space="PSUM") as p4ps_d:
            # per-k-tile weight DMAs so the first gate matmul starts after
            # 256KB instead of 4MB
            wg_t = p4w.tile([128, 16, FFr], BF16, tag="wg_t", name="wg_t")
            for k in range(16):
                nc.sync.dma_start(wg_t[:, k, :],
                                  io['w_gate'][k * 128:(k + 1) * 128, :])
            wu_t = p4w.tile([128, 16, FFr], BF16, tag="wu_t", name="wu_t")
            for k in range(16):
                nc.sync.dma_start(wu_t[:, k, :],
                                  io['w_up'][k * 128:(k + 1) * 128, :])
            wd_t = p4w.tile([128, 8, H], BF16, tag="wd_t", name="wd_t")
            for k in range(8):
                nc.sync.dma_start(wd_t[:, k, :],
                                  io['w_down'][k * 128:(k + 1) * 128, :])

            def p4_prep_a(j):
                # DMA issue + s3 scale chain only: no heavy PE work, so this
                # can sit between other matmuls without head-of-line blocking
                tok0 = j * NT
                mt = p4mt.tile([128, 16, NT], BF16, tag="mt", name="mt")
                nc.sync.dma_start(mt[:], scr['ar1_out8'][j]
                                  .rearrange("(kt p) n -> p kt n", p=128))
                ht_all = p4mt.tile([128, 16, NT], BF16, tag="ht_all",
                                   name="ht_all")
                nc.sync.dma_start(ht_all[:], io['hsT'][:, tok0:tok0 + NT]
                                  .rearrange("(kt p) n -> p kt n", p=128))
                ssq_t = p4row.tile([1, NT], F32, tag="ssq_t", name="ssq_t")
                nc.sync.dma_start(ssq_t[:], scr['ssq_out'][:, tok0:tok0 + NT])
                ssq_l = p4row.tile([1, NT], F32, tag="ssq_l", name="ssq_l")
                nc.scalar.activation(ssq_l[:], ssq_t[:], AF.Ln,
                                     bias=eps1[:], scale=float(1.0 / DIN))
                s3_row = p4row.tile([1, NT], F32R, tag="s3row", name="s3row")
                nc.scalar.activation(s3_row[:], ssq_l[:], AF.Exp, scale=-0.5)
                s3b_ps = p4ps_s.tile([128, NT], F32, tag="bps", name="s3bps")
                mm(s3b_ps[:], ones1[:], s3_row[:], start=True, stop=True)
                s3b = p4.tile([128, NT], BF16, tag="s3b", name="s3b")
                nc.any.tensor_copy(s3b[:], s3b_ps[:])
                return mt, ht_all, s3b

            def p4_prep_b(j, pa):
                # the PE-heavy stats tail, emitted AFTER the gate/up matmuls
                # so those never stall behind it in the tensor queue
                mt, ht_all, s3b = pa
                tok0 = j * NT
                ssq_ps = p4ps_s.tile([1, NT], F32, tag="ssq", name="ssq")
                for k in range(16):
                    nc.vector.tensor_tensor(mt[:, k, :], mt[:, k, :], s3b[:],
                                            Op.mult)
                    nc.vector.tensor_tensor(mt[:, k, :], mt[:, k, :],
                                            ht_all[:, k, :], Op.add)
                    nc.sync.dma_start(
                        io['resid2T'][k * 128:(k + 1) * 128, tok0:tok0 + NT],
                        mt[:, k, :])
                    sq = p4.tile([128, NT], F32R, tag="sq", name="sq")
                    nc.scalar.activation(sq[:], mt[:, k, :], AF.Square)
                    mm(ssq_ps[:], ones128[:], sq[:],
                       start=(k == 0), stop=(k == 15))
                sr0 = p4row.tile([1, NT], F32, tag="sr0", name="sr0")
                nc.scalar.activation(sr0[:], ssq_ps[:], AF.Ln,
                                     bias=eps1[:], scale=float(1.0 / H))
                s_row = p4row.tile([1, NT], F32R, tag="srow", name="srow")
                nc.scalar.activation(s_row[:], sr0[:], AF.Exp, scale=-0.5)
                sb_ps = p4ps_s.tile([128, NT], F32, tag="bps", name="sbps")
                mm(sb_ps[:], ones1[:], s_row[:], start=True, stop=True)
                sb = p4.tile([128, NT], BF16, tag="sb", name="sb")
                nc.any.tensor_copy(sb[:], sb_ps[:])
                mtn = p4.tile([128, 16, NT], BF16, tag="mtn", name="mtn")
                for k in range(16):
                    nc.vector.scalar_tensor_tensor(
                        mtn[:, k, :], mt[:, k, :], C['ln2_c'][:, k:k + 1],
                        sb[:], Op.mult, Op.mult)
                return mtn

            # tile 0's prep runs inline at phase start (no more phase-2
            # precompute: frees ~34KB of SBUF for the merged pipeline)
            mtn_cur = p4_prep_b(0, p4_prep_a(0))

            for nt in range(n8):
                tok0 = nt * NT
                # next tile's prep DMAs + s3 chain are issued FIRST so the
                # data lands while this tile's gate/up matmuls stream
                pa_next = p4_prep_a(nt + 1) if nt + 1 < n8 else None
                # gate_up + silu*up (av kept in SBUF as down-proj k-tiles)
                av = p4av.tile([128, 8, NT], BF16, tag="av", name="av")
                for mi in range(8):
                    gp = p4ps_g.tile([128, NT], F32, tag="gp", name="gp")
                    up = p4ps_g.tile([128, NT], F32, tag="up", name="up")
                    for k in range(16):
                        mm(gp[:], wg_t[:, k, mi * 128:(mi + 1) * 128],
                           mtn_cur[:, k, :], start=(k == 0), stop=(k == 15))
                    for k in range(16):
                        mm(up[:], wu_t[:, k, mi * 128:(mi + 1) * 128],
                           mtn_cur[:, k, :], start=(k == 0), stop=(k == 15))
                    sg = p4.tile([128, NT], BF16, tag="sg", name="sg")
                    silu(sg[:], gp[:], pool=p4, tag="sgsig")
                    nc.vector.tensor_tensor(av[:, mi, :], sg[:], up[:], Op.mult)
                # stats + mtn chain lands between up and down: its DVE/ACT
                # deps were satisfied during the gate/up stream, and the mtn
                # stt chain hides under the down matmuls
                mtn_next = p4_prep_b(nt + 1, pa_next) if pa_next else None
                # down proj -> ReduceScatter chunk (host concats slices).
                # The LAST tile is split into two token-halves so its first
                # RS overlaps the second half's matmuls.
                if nt < n8 - 1:
                    for mo in range(16):
                        ps = p4ps_d.tile([128, NT], F32, tag="dps", name="dps")
                        for k in range(8):
                            mm(ps[:], wd_t[:, k, mo * 128:(mo + 1) * 128],
                               av[:, k, :], start=(k == 0), stop=(k == 7))
                        ot = p4.tile([128, NT], BF16, tag="ot4", name="ot4")
                        nc.any.tensor_copy(ot[:], ps[:])
                        nc.sync.dma_start(
                            scr['rs2_in8'][nt][mo * 128:(mo + 1) * 128, :],
                            ot[:])
                    if world > 1:
                        nc.gpsimd.collective_compute(
                            "ReduceScatter", Op.add,
                            replica_groups=[list(range(world))],
                            ins=[scr['rs2_in8'][nt]],
                            outs=[scr['rs2_out8'][nt]])
                    else:
                        nc.sync.dma_start(scr['rs2_out8'][nt],
                                          scr['rs2_in8'][nt][0:H // world, :])
                    nc.sync.dma_start(io['out1T'][:, tok0:tok0 + NT],
                                      scr['rs2_out8'][nt])
                else:
                    NH2 = NT // 2
                    for half in range(2):
                        c0 = half * NH2
                        for mo in range(16):
                            ps = p4ps_d.tile([128, NH2], F32, tag="dps",
                                             name="dps")
                            for k in range(8):
                                mm(ps[:], wd_t[:, k, mo * 128:(mo + 1) * 128],
                                   av[:, k, c0:c0 + NH2],
                                   start=(k == 0), stop=(k == 7))
                            ot = p4.tile([128, NH2], BF16, tag="ot4",
                                         name="ot4")
                            nc.any.tensor_copy(ot[:], ps[:])
                            nc.sync.dma_start(
                                scr['rs2h_in'][half][mo * 128:(mo + 1) * 128,
                                                     :], ot[:])
                        if world > 1:
                            nc.gpsimd.collective_compute(
                                "ReduceScatter", Op.add,
                                replica_groups=[list(range(world))],
                                ins=[scr['rs2h_in'][half]],
                                outs=[scr['rs2h_out'][half]])
                        else:
                            nc.sync.dma_start(
                                scr['rs2h_out'][half],
                                scr['rs2h_in'][half][0:H // world, :])
                        nc.sync.dma_start(
                            io['out1T'][:, tok0 + c0:tok0 + c0 + NH2],
                            scr['rs2h_out'][half])
                mtn_cur = mtn_next


# ================================================================ entry point
def kernel(**inputs):
    from concourse import bass_utils

    nc = build(world=TP, debug=False)
    in_maps = [shard_core_inputs(inputs, r) for r in range(TP)]
    res = bass_utils.run_bass_kernel_spmd(nc, in_maps, core_ids=list(range(TP)))
    out1T = np.concatenate(
        [np.asarray(res.results[r]['out1T'], dtype=np.float32)
         for r in range(TP)], axis=0)                # [H, T] feature-major
    out1 = np.ascontiguousarray(out1T.T).reshape(B, L, H)
    resid2 = np.ascontiguousarray(
        np.asarray(res.results[0]['resid2T'], dtype=np.float32).T
    ).reshape(B, L, H)
    return out1, resid2


if __name__ == '__main__':
    nc = build(world=1)
    print("built ok")


# revision 23
# speedup vs baseline: 1.1034x; 1.0706x over previous
"""Trainium2 Bass kernel for nn_BambaMixerDecoderLayer_84696755077458.

Tensor-parallel over 8 NeuronCores (vLLM-style), v3 (merged pipeline):
  - in_proj / gate_up column-sharded, out_proj / down row-sharded
  - heads + conv channels sharded with d_inner; B/C conv streams replicated
  - SSM scan via chunked SSD (Mamba2): group-of-4 segsum built in PSUM fp32
    (batched col/tri matmuls at N=512), per-head fp32r broadcasts; bf16
    moving operands on all y matmuls (fp32 state master + bf16 shadow).
  - in_proj (tile nt) emission is INTERLEAVED with SSD chunks + gated-norm +
    out_proj of tile nt-1, so the PE stream never gaps: the tensor clock
    stays ramped and the SSD dependency chain hides under in_proj matmuls.
    Conv/x/z streams stay in SBUF (no DRAM bounce).
  - AllReduce chunked per 512-token tile and issued inline starting ~100us
    into the kernel; rmsnorm sum-of-squares rides in 2 batched fp32 ARs.
  - MLP phase: prep DMAs for tile j+1 issued before tile j's gate/up, stats
    matmuls emitted between up and down (no tensor-queue HOL), mtn chain
    hidden under down; last tile split in token halves to shrink the
    ReduceScatter tail.
Everything on-device is feature-major ([feature, token]); host does layout
transforms (transpose / shard / concat) only.

Self-contained: hardcodes all shapes; needs only /opt/trn_rl_repo on sys.path.
"""
import sys
from contextlib import ExitStack

if '/opt/trn_rl_repo' not in sys.path:
    sys.path.insert(0, '/opt/trn_rl_repo')

import numpy as np

# ---------------------------------------------------------------- constants
H = 2048          # hidden
DIN = 4096        # mamba intermediate
DS = 128          # ssm state
DCONV = 4
NH = 64
HD = 64
FF = 8192
EPS = 1e-5
B, L = 2, 2048
T = B * L                         # 4096 tokens
CONV_DIM = DIN + 2 * DS           # 4352
D_IN_PROJ = 2 * DIN + 2 * DS + NH  # 8512

TP = 8
NHr = NH // TP                    # 8 heads / core
DINr = DIN // TP                  # 512
FFr = FF // TP                    # 1024
CONVr = DINr + 2 * DS             # 768 conv channels / core
MPROJ = DINr + CONVr + NHr        # 1288 in_proj cols / core

Q = 128                           # SSD chunk
NT = 512                          # token tile (also the collective chunk)
NEG = -3.0e38
SIM_SILU = False   # True: emit sigmoid+mul instead of Silu (CoreSim support)


def _f32(x):
    return np.ascontiguousarray(np.asarray(x, dtype=np.float32))


def _bf16(x):
    import ml_dtypes
    return np.ascontiguousarray(
        np.asarray(x, dtype=np.float32).astype(ml_dtypes.bfloat16))


# ================================================================ host prep
def host_constants():
    import ml_dtypes
    identb = np.eye(128, dtype=ml_dtypes.bfloat16)
    i8 = np.eye(8, dtype=np.float32)
    sel8 = np.zeros((8, 8 * 128), np.float32)
    for h in range(8):
        sel8[h, h * 128:(h + 1) * 128] = 1.0
    # negsel4[4g+i, g*512 + i*128 + q'] = -1  (col term for group-of-4 segsum)
    negsel4 = np.zeros((8, 1024), np.float32)
    for h in range(8):
        negsel4[h, h * 128:(h + 1) * 128] = -1.0
    ones8 = np.ones((8, 128), np.float32)
    ones1 = np.ones((1, 128), np.float32)
    ones128 = np.ones((128, 1), np.float32)
    tri = np.where(np.arange(Q)[:, None] > np.arange(Q)[None, :], NEG, 0.0)
    trimask4 = np.concatenate([tri] * 4, axis=1).astype(ml_dtypes.bfloat16)
    return dict(c_identb=identb, c_i8=i8, c_sel8=sel8,
                c_sel8b=sel8.astype(ml_dtypes.bfloat16), c_negsel4=negsel4,
                c_ones8=ones8, c_ones1=ones1, c_ones128=ones128,
                c_ones128b=ones128.astype(ml_dtypes.bfloat16),
                c_trimask4=trimask4)


def shard_core_inputs(inputs, r):
    """Build the per-core input map (all feature-major)."""
    w_in = _f32(inputs['w_in'])
    zs = slice(DINr * r, DINr * (r + 1))
    xs = slice(DIN + DINr * r, DIN + DINr * (r + 1))
    bs = slice(2 * DIN, 2 * DIN + DS)
    cs = slice(2 * DIN + DS, 2 * DIN + 2 * DS)
    dts = slice(2 * DIN + 2 * DS + NHr * r, 2 * DIN + 2 * DS + NHr * (r + 1))
    w_in_r = np.concatenate(
        [w_in[:, zs], w_in[:, xs], w_in[:, bs], w_in[:, cs], w_in[:, dts]], axis=1)

    conv_w = _f32(inputs['conv_w'])
    conv_w_r = np.concatenate([conv_w[DINr * r:DINr * (r + 1)], conv_w[DIN:]], axis=0)
    conv_b = _f32(inputs['conv_b'])
    conv_b_r = np.concatenate([conv_b[DINr * r:DINr * (r + 1)], conv_b[DIN:]], axis=0)

    hs = _f32(inputs['hidden_states'])
    hs = hs.reshape(-1, H)

    A_r = _f32(inputs['A_log'])[NHr * r:NHr * (r + 1)]
    dtb_r = _f32(inputs['dt_bias'])[NHr * r:NHr * (r + 1)]
    D_r = _f32(inputs['D_ssm'])[NHr * r:NHr * (r + 1)]

    m = dict(host_constants())
    m['hsT'] = _bf16(hs.T)                                      # [2048, T]
    m['w_in'] = _bf16(w_in_r)                                   # [2048, 1288]
    # per-k-tile column form of per-feature vectors: [128, n_tiles]
    m['ln1_c'] = np.ascontiguousarray(_f32(inputs['ln1_w']).reshape(16, 128).T)
    m['ln2_c'] = np.ascontiguousarray(_f32(inputs['ln2_w']).reshape(16, 128).T)
    m['normw_c'] = np.ascontiguousarray(
        _f32(inputs['norm_w'])[DINr * r:DINr * (r + 1)].reshape(4, 128).T)
    m['dssm_c'] = np.ascontiguousarray(
        np.repeat(D_r, HD).reshape(4, 128).T)                   # [128, 4]
    # conv weights: [128, 6*4] with [p, pt*4+d]
    m['conv_w'] = np.ascontiguousarray(
        conv_w_r.reshape(6, 128, DCONV).transpose(1, 0, 2).reshape(128, 6 * DCONV))
    m['conv_b'] = np.ascontiguousarray(conv_b_r.reshape(6, 128).T)  # [128, 6]
    m['a_col'] = np.ascontiguousarray((-np.exp(A_r))[:, None])   # [8,1]
    m['dtb_col'] = np.ascontiguousarray(dtb_r[:, None])          # [8,1]
    m['w_out'] = _bf16(_f32(inputs['w_out'])[DINr * r:DINr * (r + 1)])
    wgu = _f32(inputs['w_gate_up'])
    m['w_gate'] = _bf16(wgu[:, FFr * r:FFr * (r + 1)])
    m['w_up'] = _bf16(wgu[:, FF + FFr * r:FF + FFr * (r + 1)])
    m['w_down'] = _bf16(_f32(inputs['w_down'])[FFr * r:FFr * (r + 1)])
    return m


# ================================================================ the kernel
def build(world=TP, debug=False, T_=T):
    import concourse.mybir as mybir
    import concourse.tile as tile
    from concourse import bacc
    from concourse.alu_op_type import AluOpType as Op

    AF = mybir.ActivationFunctionType
    F32 = mybir.dt.float32
    BF16 = mybir.dt.bfloat16

    nc = bacc.Bacc("TRN2", target_bir_lowering=False, debug=False,
                   num_devices=world)

    F32R = mybir.dt.float32r
    n8 = T_ // NT

    def din(name, shape, dt):
        return nc.dram_tensor(name, list(shape), dt, kind="ExternalInput").ap()

    BIN = {'hsT', 'w_in', 'w_out', 'w_gate', 'w_up', 'w_down', 'c_identb',
           'c_trimask4', 'c_sel8b', 'c_ones128b'}
    RIN = {'c_i8', 'c_sel8', 'c_negsel4', 'c_ones8', 'c_ones1', 'c_ones128'}
    io = {}
    for name, shape in [
            ('hsT', (H, T_)), ('w_in', (H, MPROJ)),
            ('ln1_c', (128, 16)), ('ln2_c', (128, 16)),
            ('normw_c', (128, 4)), ('dssm_c', (128, 4)),
            ('conv_w', (128, 24)), ('conv_b', (128, 6)),
            ('a_col', (8, 1)), ('dtb_col', (8, 1)),
            ('w_out', (DINr, H)), ('w_gate', (H, FFr)), ('w_up', (H, FFr)),
            ('w_down', (FFr, H)),
            ('c_identb', (128, 128)), ('c_i8', (8, 8)), ('c_sel8', (8, 1024)),
            ('c_sel8b', (8, 1024)), ('c_negsel4', (8, 1024)),
            ('c_ones8', (8, 128)),
            ('c_ones1', (1, 128)), ('c_ones128', (128, 1)),
            ('c_ones128b', (128, 1)),
            ('c_trimask4', (128, 512))]:
        dt = BF16 if name in BIN else (F32R if name in RIN else F32)
        io[name] = din(name, shape, dt)

    io['out1T'] = nc.dram_tensor("out1T", [H // world, T_], BF16,
                                 kind="ExternalOutput").ap()
    io['resid2T'] = nc.dram_tensor("resid2T", [H, T_], BF16,
                                   kind="ExternalOutput").ap()

    scr = {}
    scr['ar1_in8'] = [
        nc.dram_tensor(f"ar1_in{q}", [H, NT], BF16, kind="Internal").ap()
        for q in range(n8)]
    scr['ar1_out8'] = [
        nc.dram_tensor(f"ar1_out{q}", [H, NT], BF16, kind="Internal",
                       addr_space="Shared").ap() for q in range(n8)]
    scr['ssq_in'] = nc.dram_tensor("ssq_in", [1, T_], F32,
                                   kind="Internal").ap()
    half_t = min(4 * NT, T_)
    scr['ssq_outA'] = nc.dram_tensor("ssq_outA", [1, half_t], F32,
                                     kind="Internal",
                                     addr_space="Shared").ap()
    if T_ > half_t:
        scr['ssq_outB'] = nc.dram_tensor("ssq_outB", [1, T_ - half_t], F32,
                                         kind="Internal",
                                         addr_space="Shared").ap()
    scr['rs2_in8'] = [
        nc.dram_tensor(f"rs2_in{q}", [H, NT], BF16, kind="Internal").ap()
        for q in range(n8)]
    scr['rs2_out8'] = [
        nc.dram_tensor(f"rs2_out{q}", [H // world, NT], BF16,
                       kind="Internal").ap() for q in range(n8)]
    scr['rs2h_in'] = [
        nc.dram_tensor(f"rs2h_in{q}", [H, NT // 2], BF16,
                       kind="Internal").ap() for q in range(2)]
    scr['rs2h_out'] = [
        nc.dram_tensor(f"rs2h_out{q}", [H // world, NT // 2], BF16,
                       kind="Internal").ap() for q in range(2)]

    with tile.TileContext(nc) as tc:
        _body(tc, io, scr, world, debug, mybir, tile, Op, AF, F32, T_)

    nc.compile()
    return nc


def _body(tc, io, scr, world, debug, mybir, tile, Op, AF, F32, T_):
    nc = tc.nc
    F32R = mybir.dt.float32r
    BF16 = mybir.dt.bfloat16
    n8 = T_ // NT
    NCHUNK = T_ // Q
    CPS = (T_ // B) // Q          # chunks per sequence

    def mm(out, lhsT, rhs, start, stop, skip=False):
        if lhsT.dtype == F32:
            lhsT = lhsT.bitcast(F32R)
        if rhs.dtype == F32:
            rhs = rhs.bitcast(F32R)
        nc.tensor.matmul(out, lhsT, rhs, start=start, stop=stop,
                         skip_group_check=skip)

    def silu(out_ap, in_ap, bias=0.0, pool=None, tag="silu_tmp"):
        if SIM_SILU:
            tmp = pool.tile(list(out_ap.shape), F32, tag=tag, name=tag)
            nc.scalar.activation(tmp[:], in_ap, AF.Sigmoid, bias=bias, scale=1.0)
            if isinstance(bias, float) and bias == 0.0:
                nc.vector.tensor_tensor(out_ap, in_ap, tmp[:], Op.mult)
            else:
                raise NotImplementedError("SIM_SILU with bias AP")
        else:
            nc.scalar.activation(out_ap, in_ap, AF.Silu, bias=bias, scale=1.0)

    def allreduce(in_ap, out_ap):
        if world > 1:
            nc.gpsimd.collective_compute(
                "AllReduce", Op.add, replica_groups=[list(range(world))],
                ins=[in_ap], outs=[out_ap])
        else:
            nc.sync.dma_start(out_ap, in_ap)

    with ExitStack() as ES:
        cpool = ES.enter_context(tc.tile_pool(name="consts", bufs=1))

        # -------------------------------------------------------- constants
        C = {}
        RT = {'c_i8', 'c_sel8', 'c_negsel4', 'c_ones8', 'c_ones1',
              'c_ones128'}
        BT = {'c_identb', 'c_trimask4', 'c_sel8b', 'c_ones128b'}
        for nm, shape in [('c_identb', (128, 128)), ('c_i8', (8, 8)),
                          ('c_sel8', (8, 1024)), ('c_sel8b', (8, 1024)),
                          ('c_negsel4', (8, 1024)),
                          ('c_ones8', (8, 128)), ('c_ones1', (1, 128)),
                          ('c_ones128', (128, 1)), ('c_ones128b', (128, 1)),
                          ('c_trimask4', (128, 512)),
                          ('ln1_c', (128, 16)), ('ln2_c', (128, 16)),
                          ('normw_c', (128, 4)), ('dssm_c', (128, 4)),
                          ('conv_w', (128, 24)), ('conv_b', (128, 6)),
                          ('a_col', (8, 1)), ('dtb_col', (8, 1))]:
            dt = BF16 if nm in BT else (F32R if nm in RT else F32)
            t = cpool.tile(list(shape), dt, tag=nm)
            nc.sync.dma_start(t[:], io[nm])
            C[nm] = t
        identb, i8 = C['c_identb'], C['c_i8']
        sel8, negsel4 = C['c_sel8'], C['c_negsel4']
        sel8b = C['c_sel8b']
        ones8, ones1, ones128 = C['c_ones8'], C['c_ones1'], C['c_ones128']
        ones128b = C['c_ones128b']
        trimask4 = C['c_trimask4']

        eps1 = cpool.tile([1, 1], F32, tag="eps1", name="eps1")
        nc.vector.memset(eps1[:], float(EPS))

        # ============================== merged in_proj + conv + SSD + out
        with tc.tile_pool(name="p1w", bufs=1) as p1w, \
             tc.tile_pool(name="p1h", bufs=2) as p1h, \
             tc.tile_pool(name="sxz", bufs=2) as sxz, \
             tc.tile_pool(name="cvt", bufs=2) as cvt, \
             tc.tile_pool(name="cvx", bufs=1) as cvx, \
             tc.tile_pool(name="p2s", bufs=2) as p2s, \
             tc.tile_pool(name="p1s", bufs=2) as p1s, \
             tc.tile_pool(name="pdt", bufs=1) as pdt, \
             tc.tile_pool(name="prow", bufs=2) as prow, \
             tc.tile_pool(name="spool", bufs=1) as spool, \
             tc.tile_pool(name="p3w", bufs=1) as p3w, \
             tc.tile_pool(name="psM", bufs=2, space="PSUM") as psM, \
             tc.tile_pool(name="psS", bufs=1, space="PSUM") as psS, \
             tc.tile_pool(name="psC", bufs=1, space="PSUM") as psC, \
             tc.tile_pool(name="psG", bufs=2, space="PSUM") as psG:

            # hst tile 0 prefetched before weights so stats matmuls can warm
            # up the PE while w_in streams in
            hst0 = p1h.tile([128, 16, NT], BF16, tag="hst", name="hst")
            nc.sync.dma_start(hst0[:, 0:8, :], io['hsT'][0:8 * 128, 0:NT]
                              .rearrange("(kt p) n -> p kt n", p=128))
            nc.sync.dma_start(hst0[:, 8:16, :], io['hsT'][8 * 128:H, 0:NT]
                              .rearrange("(kt p) n -> p kt n", p=128))
            w1 = p1w.tile([128, 16, MPROJ], BF16, tag="w1", name="w1")
            nc.sync.dma_start(
                w1[:], io['w_in'].rearrange("(kt p) m -> p kt m", p=128))
            for k in range(16):
                nc.vector.tensor_scalar_mul(w1[:, k, :], w1[:, k, :],
                                            C['ln1_c'][:, k:k + 1])
            w_out_t = p3w.tile([128, 4, H], BF16, tag="w_out_t",
                               name="w_out_t")
            nc.sync.dma_start(w_out_t[:],
                              io['w_out'].rearrange("(kt p) m -> p kt m",
                                                    p=128))
            for k in range(4):
                nc.vector.tensor_scalar_mul(w_out_t[:, k, :], w_out_t[:, k, :],
                                            C['normw_c'][:, k:k + 1])

            S_all = spool.tile([128, NHr * HD], F32R, tag="S_all",
                               name="S_all")
            nc.vector.memset(S_all[:].bitcast(F32), 0.0)
            # bf16 shadow of the state for the y_inter matmul (PE cannot mix
            # f32r weights with bf16 moving); fp32 master stays exact
            S_b = spool.tile([128, NHr * HD], BF16, tag="S_b", name="S_b")
            nc.vector.memset(S_b[:], 0.0)

            # ---------------- per-tile emit pieces -------------------------
            TS = {}           # state shared between pieces of one tile

            def emit_stats(nt):
                """ln1 sum-of-squares + rms scale for tile nt."""
                tok0 = nt * NT
                if nt == 0:
                    hst = hst0
                else:
                    hst = p1h.tile([128, 16, NT], BF16, tag="hst", name="hst")
                    nc.sync.dma_start(hst[:], io['hsT'][:, tok0:tok0 + NT]
                                      .rearrange("(kt p) n -> p kt n", p=128))
                sm = psS.tile([128, NT], F32, tag="small", name="small")
                ssq_ps = sm[0:1, :]
                for k in range(16):
                    sq = p1s.tile([128, NT], BF16, tag="sq", name="sq")
                    nc.scalar.activation(sq[:], hst[:, k, :], AF.Square)
                    mm(ssq_ps, ones128b[:], sq[:],
                       start=(k == 0), stop=(k == 15), skip=True)
                sr0 = pdt.tile([1, NT], F32, tag="sr0", name="sr0")
                nc.scalar.activation(sr0[:], ssq_ps, AF.Ln,
                                     bias=eps1[:], scale=float(1.0 / H))
                s_row = p1s.tile([1, NT], F32R, tag="s_row", name="s_row")
                nc.scalar.activation(s_row[:], sr0[:], AF.Exp, scale=-0.5)
                sb_ps = psM.tile([128, NT], F32, tag="mt", name="sbps")
                mm(sb_ps[:], ones1[:], s_row[:], start=True, stop=True)
                sb = p1s.tile([128, NT], F32, tag="sb", name="sb")
                nc.any.tensor_copy(sb[:], sb_ps[:])
                TS['hst'], TS['sb'] = hst, sb
                TS['z_sb'] = sxz.tile([128, 4, NT], BF16, tag="z_sb",
                                      name="z_sb")
                TS['xbc'] = cvx.tile([128, 6, NT + 3], BF16, tag="xbc",
                                     name="xbc")
                TS['xconv'] = sxz.tile([128, 6, NT], BF16, tag="xconv",
                                       name="xconv")
                TS['tail'] = cvt.tile([128, 6, 3], BF16, tag="cvtail",
                                      name="cvtail")

            def emit_mz(nt, mi):
                """z m-tile mi of tile nt."""
                hst, sb = TS['hst'], TS['sb']
                ps = psM.tile([128, NT], F32, tag="mt", name="mt")
                for k in range(16):
                    mm(ps[:], w1[:, k, mi * 128:(mi + 1) * 128],
                       hst[:, k, :], start=(k == 0), stop=(k == 15))
                nc.vector.tensor_tensor(TS['z_sb'][:, mi, :], ps[:], sb[:],
                                        Op.mult)

            def emit_mxbc(nt, pt, tail_prev):
                """xBC m-tile pt of tile nt + causal conv + silu."""
                tok0 = nt * NT
                seq_start = (tok0 % (T_ // B)) == 0
                hst, sb = TS['hst'], TS['sb']
                xbc, xconv = TS['xbc'], TS['xconv']
                ps = psM.tile([128, NT], F32, tag="mt", name="mt")
                for k in range(16):
                    mm(ps[:], w1[:, k, DINr + pt * 128:DINr + (pt + 1) * 128],
                       hst[:, k, :], start=(k == 0), stop=(k == 15))
                nc.vector.tensor_tensor(xbc[:, pt, 3:3 + NT], ps[:], sb[:],
                                        Op.mult)
                if seq_start:
                    nc.vector.memset(xbc[:, pt, 0:3], 0.0)
                else:
                    nc.vector.tensor_copy(xbc[:, pt, 0:3],
                                          tail_prev[:, pt, :])
                acc = p1s.tile([128, NT], BF16, tag="cacc", name="cacc")
                nc.vector.tensor_scalar_mul(
                    acc[:], xbc[:, pt, 0:NT],
                    C['conv_w'][:, pt * 4:pt * 4 + 1])
                for dd in range(1, 4):
                    nc.vector.scalar_tensor_tensor(
                        acc[:], xbc[:, pt, dd:dd + NT],
                        C['conv_w'][:, pt * 4 + dd:pt * 4 + dd + 1],
                        acc[:], Op.mult, Op.add)
                if SIM_SILU:
                    nc.vector.tensor_scalar_add(acc[:], acc[:],
                                                C['conv_b'][:, pt:pt + 1])
                    silu(xconv[:, pt, :], acc[:], pool=p1s, tag="cvsig")
                else:
                    nc.scalar.activation(xconv[:, pt, :], acc[:], AF.Silu,
                                         bias=C['conv_b'][:, pt:pt + 1],
                                         scale=1.0)
                nc.vector.tensor_copy(TS['tail'][:, pt, :],
                                      xbc[:, pt, NT:NT + 3])

            def emit_dt(nt):
                """dt m-tile + softplus + per-chunk log-decay cumsum rows."""
                hst, sb = TS['hst'], TS['sb']
                sm = psS.tile([128, NT], F32, tag="small", name="small")
                dtp = sm[32:40, :]
                for k in range(16):
                    mm(dtp, w1[:, k, DINr + CONVr:MPROJ], hst[:, k, :],
                       start=(k == 0), stop=(k == 15), skip=True)
                dt_raw = pdt.tile([8, NT], F32, tag="dtraw", name="dtraw")
                nc.vector.tensor_tensor(dt_raw[:], dtp, sb[:8, :], Op.mult)
                e8 = pdt.tile([8, NT], F32, tag="e8", name="e8")
                nc.scalar.activation(e8[:], dt_raw[:], AF.Exp,
                                     bias=C['dtb_col'][:], scale=1.0)
                nc.vector.tensor_scalar_add(e8[:], e8[:], 1.0)
                dtr = prow.tile([8, NT], F32R, tag="dtr", name="dtr")
                nc.scalar.activation(dtr[:], e8[:], AF.Ln)
                logda = dt_raw
                nc.vector.tensor_scalar_mul(logda[:], dtr[:], C['a_col'][:])
                lar = prow.tile([8, NT], F32R, tag="lar", name="lar")
                for c in range(NT // Q):
                    nc.vector.tensor_tensor_scan(
                        lar[:, c * Q:(c + 1) * Q],
                        ones8[:, :Q].bitcast(F32), logda[:, c * Q:(c + 1) * Q],
                        0.0, Op.mult, Op.add)
                TS['dtr'], TS['lar'] = dtr, lar

            # ---- SSD chunk pieces (operate on tile data in CS dict) ------
            def chunk_front(CS, ch):
                """misc + transposes + segsum matmuls for chunk ch."""
                cc_ = ((ch * Q) % NT) // Q
                xconv = CS['xconv']
                xf = xconv[:, 0:4, cc_ * Q:(cc_ + 1) * Q]
                bf = xconv[:, 4, cc_ * Q:(cc_ + 1) * Q]
                cf = xconv[:, 5, cc_ * Q:(cc_ + 1) * Q]
                lrow = CS['lar'][:, cc_ * Q:(cc_ + 1) * Q]
                dtrow = CS['dtr'][:, cc_ * Q:(cc_ + 1) * Q]

                explb = p2s.tile([8, Q], BF16, tag="explb", name="explb")
                nc.scalar.activation(explb[:], lrow, AF.Exp)
                expl_l = p2s.tile([8, 1], F32, tag="expl_l", name="expl_l")
                nc.scalar.activation(expl_l[:], lrow[:, Q - 1:Q], AF.Exp)
                ddr0 = p2s.tile([8, Q], F32, tag="ddr0", name="ddr0")
                nc.vector.tensor_scalar(ddr0[:], lrow, -1.0,
                                        lrow[:, Q - 1:Q].bitcast(F32),
                                        Op.mult, Op.add)
                dd_rows = p2s.tile([8, Q], F32R, tag="ddrows", name="ddrows")
                nc.scalar.activation(dd_rows[:], ddr0[:], AF.Exp)
                nc.vector.tensor_tensor(dd_rows[:], dd_rows[:], dtrow,
                                        Op.mult)
                dg = p2s.tile([8, 8], F32R, tag="dg", name="dg")
                nc.vector.tensor_scalar_mul(dg[:], i8[:], expl_l[:])

                mix = psC.tile([128, 480], F32, tag="mix", name="mix")
                misc = mix[:, 0:160]
                tpsb = mix[:, 160:480].bitcast(BF16)     # [128, 640] bf16
                mm(misc[:, 0:128], bf, cf, start=True, stop=True, skip=True)
                mm(misc[:, 128:136], dd_rows[:], i8[:], start=True,
                   stop=True, skip=True)
                mm(misc[:, 136:144], ones8[:], dg[:], start=True, stop=True,
                   skip=True)
                mm(misc[:, 144:152], dtrow, i8[:], start=True, stop=True,
                   skip=True)
                g_sb = p2s.tile([128, 128], BF16, tag="g_sb", name="g_sb")
                nc.any.tensor_copy(g_sb[:], misc[:, 0:128])
                dsc = p2s.tile([128, 16], F32, tag="dsc", name="dsc")
                nc.any.tensor_copy(dsc[:], misc[:, 128:144])
                dtc_b = p2s.tile([128, 8], BF16, tag="dtc_b", name="dtc_b")
                nc.any.tensor_copy(dtc_b[:], misc[:, 144:152])

                nc.tensor.transpose(tpsb[:, 512:640], bf, identb[:])
                btm = p2s.tile([128, Q], BF16, tag="btm", name="btm")
                nc.any.tensor_copy(btm[:], tpsb[:, 512:640])
                for pt in range(4):
                    nc.tensor.transpose(tpsb[:, pt * 128:(pt + 1) * 128],
                                        xf[:, pt, :], identb[:])
                xtm = p2s.tile([128, NHr, HD], BF16, tag="xtm_sb",
                               name="xtm_sb")
                nc.any.tensor_copy(
                    xtm[:], tpsb[:, 0:512].rearrange(
                        "p (h d) -> p h d", d=HD))
                xw = p2s.tile([128, NHr, HD], BF16, tag="xw", name="xw")
                nc.vector.tensor_tensor(
                    xw[:], xtm[:],
                    dsc[:, 0:8][:, :, None].broadcast_to([128, NHr, HD]),
                    Op.mult)

                segs = []
                for g in range(2):
                    seg = psG.tile([128, 2, 4, 128], F32, tag="seg",
                                   name="seg")
                    segs.append(seg)
                    # PSUM zero-region semantics: start=True re-marks the
                    # whole 2KB bank pending-zero, so exactly ONE start per
                    # bank; later first-touch writes overwrite via pending
                    # bits and full-width writes accumulate.
                    for i in range(4):
                        h = 4 * g + i
                        mm(seg[:, 0, i, :],
                           sel8[:, h * 128:(h + 1) * 128], lrow,
                           start=(i == 0), stop=False, skip=True)
                    mm(seg[:, 0], lrow,
                       negsel4[:, g * 512:(g + 1) * 512],
                       start=False, stop=False, skip=True)
                    mm(seg[:, 0], identb[:], trimask4[:],
                       start=False, stop=True, skip=True)
                    for i in range(4):
                        h = 4 * g + i
                        mm(seg[:, 1, i, :],
                           sel8b[:, h * 128:(h + 1) * 128], explb[:],
                           start=(i == 0), stop=(i == 3), skip=True)
                CS.update(segs=segs, xf=xf, cf=cf, xtm=xtm, xw=xw, btm=btm,
                          g_sb=g_sb, dsc=dsc, dtc_b=dtc_b)

            def chunk_back(CS, ch):
                """w0/wt/ce + y matmuls + readout + state update."""
                cc_ = ((ch * Q) % NT) // Q
                segs, xf, cf = CS['segs'], CS['xf'], CS['cf']
                xtm, xw, btm = CS['xtm'], CS['xw'], CS['btm']
                g_sb, dsc, dtc_b = CS['g_sb'], CS['dsc'], CS['dtc_b']
                y_sb = CS['y_sb']
                for g in range(2):
                    seg = segs[g]
                    w0 = p2s.tile([128, 4, 128], BF16, tag="w0", name="w0")
                    nc.scalar.activation(w0[:], seg[:, 0], AF.Exp)
                    wt = p2s.tile([128, 4, 128], BF16, tag="wt", name="wt")
                    nc.vector.tensor_tensor(
                        wt[:], w0[:],
                        dtc_b[:, 4 * g:4 * g + 4, None]
                        .broadcast_to([128, 4, 128]), Op.mult)
                    nc.vector.tensor_tensor(
                        wt[:], wt[:],
                        g_sb[:, None, :].broadcast_to([128, 4, 128]),
                        Op.mult)
                    ce = p2s.tile([128, 4, 128], BF16, tag="ce", name="ce")
                    nc.vector.tensor_tensor(
                        ce[:], seg[:, 1],
                        cf[:, None, :].broadcast_to([128, 4, 128]),
                        Op.mult)
                    # y accumulates into the (already consumed) dquad bank:
                    # head 4g+2j -> partitions 0:64, col block j;
                    # head 4g+2j+1 -> partitions 64:128, col block j
                    for i in range(4):
                        h = 4 * g + i
                        j, half = i // 2, i % 2
                        ysl = seg[64 * half:64 * (half + 1), 0, j, :]
                        mm(ysl, xtm[:, h, :], wt[:, i, :],
                           start=True, stop=False, skip=True)
                        mm(ysl, S_b[:, h * HD:(h + 1) * HD],
                           ce[:, i, :], start=False, stop=True, skip=True)
                    for j in range(2):
                        pt = 2 * g + j
                        nc.vector.scalar_tensor_tensor(
                            y_sb[:, pt, cc_ * Q:(cc_ + 1) * Q],
                            xf[:, pt, :], C['dssm_c'][:, pt:pt + 1],
                            seg[:, 0, j, :], Op.mult, Op.add)

                tp_ps = psM.tile([128, 512], F32, tag="mt", name="tp")
                mm(tp_ps[:], btm[:], xw[:], start=True, stop=True)
                S3 = S_all[:].rearrange("p (h d) -> p h d", d=HD)
                nc.vector.tensor_tensor(
                    S3, S3,
                    dsc[:, 8:16][:, :, None].broadcast_to([128, NHr, HD]),
                    Op.mult)
                nc.vector.tensor_tensor(
                    S3, S3, tp_ps[:].rearrange("p (h d) -> p h d", d=HD),
                    Op.add)
                if (ch + 1) % CPS == 0 and ch + 1 < NCHUNK:
                    nc.vector.memset(S_all[:].bitcast(F32), 0.0)
                if ch + 1 < NCHUNK:
                    nc.vector.tensor_copy(S_b[:], S_all[:])

            def emit_p3(CS, pnt):
                """gated norm + stats + out_proj + inline AR for tile pnt."""
                tok0 = pnt * NT
                y_sb, z_sb = CS['y_sb'], CS['z_sb']
                sm = psS.tile([128, NT], F32, tag="small", name="small3")
                ssq_ps = sm[0:1, :]
                for pt in range(4):
                    silu(z_sb[:, pt, :], z_sb[:, pt, :], pool=p1s, tag="szsig")
                    nc.vector.tensor_tensor(y_sb[:, pt, :], y_sb[:, pt, :],
                                            z_sb[:, pt, :], Op.mult)
                    sqz = p1s.tile([128, NT], BF16, tag="sqz", name="sqz")
                    nc.scalar.activation(sqz[:], y_sb[:, pt, :], AF.Square)
                    mm(ssq_ps, ones128b[:], sqz[:],
                       start=(pt == 0), stop=(pt == 3), skip=True)
                sst = p1s.tile([1, NT], F32, tag="sst", name="sst")
                nc.any.tensor_copy(sst[:], ssq_ps)
                nc.sync.dma_start(scr['ssq_in'][:, tok0:tok0 + NT], sst[:])

                for mi in range(16):
                    ps = psM.tile([128, NT], F32, tag="mt", name="mt")
                    for k in range(4):
                        mm(ps[:], w_out_t[:, k, mi * 128:(mi + 1) * 128],
                           y_sb[:, k, :], start=(k == 0), stop=(k == 3))
                    ot = p1s.tile([128, NT], BF16, tag="ot", name="ot")
                    nc.any.tensor_copy(ot[:], ps[:])
                    nc.sync.dma_start(
                        scr['ar1_in8'][pnt][mi * 128:(mi + 1) * 128, :],
                        ot[:])
                allreduce(scr['ar1_in8'][pnt], scr['ar1_out8'][pnt])
                # batched rms-stats ARs: [tiles 0..3] and [tiles 4..7]
                # (separate OUT tensors: DRAM deps are tensor-granular, and
                # phase 4's tile-0 prep must not wait for the late batch)
                half_t = min(4 * NT, T_)
                if pnt == min(3, n8 - 1):
                    allreduce(scr['ssq_in'][:, 0:half_t], scr['ssq_outA'])
                if pnt == n8 - 1 and T_ > half_t:
                    allreduce(scr['ssq_in'][:, half_t:T_], scr['ssq_outB'])

            # ------------------------- merged schedule --------------------
            prev = None       # CS dict of tile nt-1 (chunks pending)
            tail_prev = None
            for nt in range(n8 + 1):
                if nt < n8:
                    emit_stats(nt)
                    mp = ([lambda mi=mi: emit_mz(nt, mi) for mi in range(4)]
                          + [lambda pt=pt, tp_=tail_prev:
                             emit_mxbc(nt, pt, tp_) for pt in range(6)]
                          + [lambda: emit_dt(nt)])
                else:
                    mp = []
                if prev is not None:
                    CS = prev
                    pnt = nt - 1
                    cps = []
                    for c in range(4):
                        ch = pnt * 4 + c
                        cps.append(lambda ch=ch: chunk_front(CS, ch))
                        cps.append(lambda ch=ch: chunk_back(CS, ch))
                    cps.append(lambda: emit_p3(CS, pnt))
                else:
                    cps = []
                # interleave: chunk piece, m piece, chunk piece, m piece ...
                ia, ib = 0, 0
                while ia < len(cps) or ib < len(mp):
                    if ia < len(cps):
                        cps[ia]()
                        ia += 1
                    if ib < len(mp):
                        mp[ib]()
                        ib += 1
                    if (ib < len(mp)
                            and len(mp) - ib > 2 * (len(cps) - ia)):
                        mp[ib]()
                        ib += 1
                if nt < n8:
                    prev = dict(xconv=TS['xconv'], z_sb=TS['z_sb'],
                                dtr=TS['dtr'], lar=TS['lar'],
                                y_sb=sxz.tile([128, 4, NT], BF16, tag="y_sb",
                                              name="y_sb"))
                    tail_prev = TS['tail']

        # ================================= Phase 4: resid + ln2 + MLP + RS
        with tc.tile_pool(name="p4w", bufs=1) as p4w, \
             tc.tile_pool(name="p4", bufs=2) as p4, \
             tc.tile_pool(name="p4mt", bufs=1) as p4mt, \
             tc.tile_pool(name="p4row", bufs=1) as p4row, \
             tc.tile_pool(name="p4av", bufs=1) as p4av, \
             tc.tile_pool(name="p4ps_s", bufs=1, space="PSUM") as p4ps_s, \
             tc.tile_pool(name="p4ps_g", bufs=2, space="PSUM") as p4ps_g, \
             tc.tile_pool(name="p4ps_d", bufs=2, space="PSUM") as p4ps_d:
            # per-k-tile weight DMAs so the first gate matmul starts after
            # 256KB instead of 4MB
            wg_t = p4w.tile([128, 16, FFr], BF16, tag="wg_t", name="wg_t")
            for k in range(16):
                nc.sync.dma_start(wg_t[:, k, :],
                                  io['w_gate'][k * 128:(k + 1) * 128, :])
            wu_t = p4w.tile([128, 16, FFr], BF16, tag="wu_t", name="wu_t")
            for k in range(16):
                nc.sync.dma_start(wu_t[:, k, :],
                                  io['w_up'][k * 128:(k + 1) * 128, :])
            wd_t = p4w.tile([128, 8, H], BF16, tag="wd_t", name="wd_t")
            for k in range(8):
                nc.sync.dma_start(wd_t[:, k, :],
                                  io['w_down'][k * 128:(k + 1) * 128, :])

            def p4_prep_a(j):
                # DMA issue + s3 scale chain only: no heavy PE work, so this
                # can sit between other matmuls without head-of-line blocking
                tok0 = j * NT
                mt = p4mt.tile([128, 16, NT], BF16, tag="mt", name="mt")
                nc.sync.dma_start(mt[:], scr['ar1_out8'][j]
                                  .rearrange("(kt p) n -> p kt n", p=128))
                ht_all = p4mt.tile([128, 16, NT], BF16, tag="ht_all",
                                   name="ht_all")
                nc.sync.dma_start(ht_all[:], io['hsT'][:, tok0:tok0 + NT]
                                  .rearrange("(kt p) n -> p kt n", p=128))
                ssq_t = p4row.tile([1, NT], F32, tag="ssq_t", name="ssq_t")
                half_t = min(4 * NT, T_)
                if tok0 < half_t:
                    nc.sync.dma_start(ssq_t[:],
                                      scr['ssq_outA'][:, tok0:tok0 + NT])
                else:
                    nc.sync.dma_start(
                        ssq_t[:],
                        scr['ssq_outB'][:, tok0 - half_t:tok0 - half_t + NT])
                ssq_l = p4row.tile([1, NT], F32, tag="ssq_l", name="ssq_l")
                nc.scalar.activation(ssq_l[:], ssq_t[:], AF.Ln,
                                     bias=eps1[:], scale=float(1.0 / DIN))
                s3_row = p4row.tile([1, NT], F32R, tag="s3row", name="s3row")
                nc.scalar.activation(s3_row[:], ssq_l[:], AF.Exp, scale=-0.5)
                s3b_ps = p4ps_s.tile([128, NT], F32, tag="bps", name="s3bps")
                mm(s3b_ps[:], ones1[:], s3_row[:], start=True, stop=True)
                s3b = p4.tile([128, NT], BF16, tag="s3b", name="s3b")
                nc.any.tensor_copy(s3b[:], s3b_ps[:])
                return mt, ht_all, s3b

            def p4_prep_b(j, pa):
                # the PE-heavy stats tail, emitted AFTER the gate/up matmuls
                # so those never stall behind it in the tensor queue
                mt, ht_all, s3b = pa
                tok0 = j * NT
                ssq_ps = p4ps_s.tile([1, NT], F32, tag="ssq", name="ssq")
                for k in range(16):
                    nc.vector.tensor_tensor(mt[:, k, :], mt[:, k, :], s3b[:],
                                            Op.mult)
                    nc.vector.tensor_tensor(mt[:, k, :], mt[:, k, :],
                                            ht_all[:, k, :], Op.add)
                    nc.sync.dma_start(
                        io['resid2T'][k * 128:(k + 1) * 128, tok0:tok0 + NT],
                        mt[:, k, :])
                    sq = p4.tile([128, NT], F32R, tag="sq", name="sq")
                    nc.scalar.activation(sq[:], mt[:, k, :], AF.Square)
                    mm(ssq_ps[:], ones128[:], sq[:],
                       start=(k == 0), stop=(k == 15))
                sr0 = p4row.tile([1, NT], F32, tag="sr0", name="sr0")
                nc.scalar.activation(sr0[:], ssq_ps[:], AF.Ln,
                                     bias=eps1[:], scale=float(1.0 / H))
                s_row = p4row.tile([1, NT], F32R, tag="srow", name="srow")
                nc.scalar.activation(s_row[:], sr0[:], AF.Exp, scale=-0.5)
                sb_ps = p4ps_s.tile([128, NT], F32, tag="bps", name="sbps")
                mm(sb_ps[:], ones1[:], s_row[:], start=True, stop=True)
                sb = p4.tile([128, NT], BF16, tag="sb", name="sb")
                nc.any.tensor_copy(sb[:], sb_ps[:])
                mtn = p4.tile([128, 16, NT], BF16, tag="mtn", name="mtn")
                for k in range(16):
                    nc.vector.scalar_tensor_tensor(
                        mtn[:, k, :], mt[:, k, :], C['ln2_c'][:, k:k + 1],
                        sb[:], Op.mult, Op.mult)
                return mtn

            # tile 0's prep runs inline at phase start (no more phase-2
            # precompute: frees ~34KB of SBUF for the merged pipeline)
            mtn_cur = p4_prep_b(0, p4_prep_a(0))

            for nt in range(n8):
                tok0 = nt * NT
                # next tile's prep DMAs + s3 chain are issued FIRST so the
                # data lands while this tile's gate/up matmuls stream
                pa_next = p4_prep_a(nt + 1) if nt + 1 < n8 else None
                # gate_up + silu*up (av kept in SBUF as down-proj k-tiles)
                av = p4av.tile([128, 8, NT], BF16, tag="av", name="av")
                for mi in range(8):
                    gp = p4ps_g.tile([128, NT], F32, tag="gp", name="gp")
                    up = p4ps_g.tile([128, NT], F32, tag="up", name="up")
                    for k in range(16):
                        mm(gp[:], wg_t[:, k, mi * 128:(mi + 1) * 128],
                           mtn_cur[:, k, :], start=(k == 0), stop=(k == 15))
                    for k in range(16):
                        mm(up[:], wu_t[:, k, mi * 128:(mi + 1) * 128],
                           mtn_cur[:, k, :], start=(k == 0), stop=(k == 15))
                    sg = p4.tile([128, NT], BF16, tag="sg", name="sg")
                    silu(sg[:], gp[:], pool=p4, tag="sgsig")
                    nc.vector.tensor_tensor(av[:, mi, :], sg[:], up[:], Op.mult)
                # stats + mtn chain lands between up and down: its DVE/ACT
                # deps were satisfied during the gate/up stream, and the mtn
                # stt chain hides under the down matmuls
                mtn_next = p4_prep_b(nt + 1, pa_next) if pa_next else None
                # down proj -> ReduceScatter chunk (host concats slices).
                # The LAST tile is split into two token-halves so its first
                # RS overlaps the second half's matmuls.
                if nt < n8 - 1:
                    for mo in range(16):
                        ps = p4ps_d.tile([128, NT], F32, tag="dps", name="dps")
                        for k in range(8):
                            mm(ps[:], wd_t[:, k, mo * 128:(mo + 1) * 128],
                               av[:, k, :], start=(k == 0), stop=(k == 7))
                        ot = p4.tile([128, NT], BF16, tag="ot4", name="ot4")
                        nc.any.tensor_copy(ot[:], ps[:])
                        nc.sync.dma_start(
                            scr['rs2_in8'][nt][mo * 128:(mo + 1) * 128, :],
                            ot[:])
                    if world > 1:
                        nc.gpsimd.collective_compute(
                            "ReduceScatter", Op.add,
                            replica_groups=[list(range(world))],
                            ins=[scr['rs2_in8'][nt]],
                            outs=[scr['rs2_out8'][nt]])
                    else:
                        nc.sync.dma_start(scr['rs2_out8'][nt],
                                          scr['rs2_in8'][nt][0:H // world, :])
                    nc.scalar.dma_start(io['out1T'][:, tok0:tok0 + NT],
                                        scr['rs2_out8'][nt])
                else:
                    NH2 = NT // 2
                    for half in range(2):
                        c0 = half * NH2
                        for mo in range(16):
                            ps = p4ps_d.tile([128, NH2], F32, tag="dps",
                                             name="dps")
                            for k in range(8):
                                mm(ps[:], wd_t[:, k, mo * 128:(mo + 1) * 128],
                                   av[:, k, c0:c0 + NH2],
                                   start=(k == 0), stop=(k == 7))
                            ot = p4.tile([128, NH2], BF16, tag="ot4",
                                         name="ot4")
                            nc.any.tensor_copy(ot[:], ps[:])
                            nc.sync.dma_start(
                                scr['rs2h_in'][half][mo * 128:(mo + 1) * 128,
                                                     :], ot[:])
                        if world > 1:
                            nc.gpsimd.collective_compute(
                                "ReduceScatter", Op.add,
                                replica_groups=[list(range(world))],
                                ins=[scr['rs2h_in'][half]],
                                outs=[scr['rs2h_out'][half]])
                        else:
                            nc.sync.dma_start(
                                scr['rs2h_out'][half],
                                scr['rs2h_in'][half][0:H // world, :])
                        nc.scalar.dma_start(
                            io['out1T'][:, tok0 + c0:tok0 + c0 + NH2],
                            scr['rs2h_out'][half])
                mtn_cur = mtn_next


# ================================================================ entry point
def kernel(**inputs):
    from concourse import bass_utils

    nc = build(world=TP, debug=False)
    in_maps = [shard_core_inputs(inputs, r) for r in range(TP)]
    res = bass_utils.run_bass_kernel_spmd(nc, in_maps, core_ids=list(range(TP)))
    out1T = np.concatenate(
        [np.asarray(res.results[r]['out1T'], dtype=np.float32)
         for r in range(TP)], axis=0)                # [H, T] feature-major
    out1 = np.ascontiguousarray(out1T.T).reshape(B, L, H)
    resid2 = np.ascontiguousarray(
        np.asarray(res.results[0]['resid2T'], dtype=np.float32).T
    ).reshape(B, L, H)
    return out1, resid2


if __name__ == '__main__':
    nc = build(world=1)
    print("built ok")
